# revision 1
# baseline (speedup 1.0000x reference)
"""Self-contained TRN2 Bass kernel for causal multi-head attention.

Problem: x[4,2048,1024], causal mask, wq/wk/wv/wo [1024,1024], H=16, HD=64.
Sharding: 8 NeuronCores = 4 batches x 2 head-groups (8 heads each).
Each core computes Q/K/V projections for its head group, causal attention
(block-skipped via host mask classification), and a partial o_proj; the two
partial outputs per batch are summed on host (the tensor-parallel
all-reduce of the unshard step).
"""
import sys
sys.path.insert(0, "/opt/trn_rl_repo")

import numpy as np

from contextlib import ExitStack

import concourse.bass as bass
import concourse.mybir as mybir
import concourse.tile as tile
from concourse import bacc

f32 = mybir.dt.float32
f32r = mybir.dt.float32r
EXP = mybir.ActivationFunctionType.Exp


def build(T=2048, C=1024, HL=8, D=64, kinds=None, mixidx=None, nmix=0, n_cores=8,
          debug=False, chunk=2):
    OL = HL * D
    JT = T // 128
    IBN = T // 512
    NC = C // 128
    NO = OL // 128
    NTB = T // 512
    NP = HL // 2
    SCALE = 1.0 / float(D) ** 0.5
    E = D + 1  # v columns per head incl. ones column
    CHUNK = chunk

    assert kinds is not None

    nc = bacc.Bacc("TRN2", target_bir_lowering=False, debug=False,
                   num_devices=n_cores)

    xT = nc.dram_tensor("xT", [C, T], f32, kind="ExternalInput").ap()
    wqT = nc.dram_tensor("wqT", [C, OL], f32, kind="ExternalInput").ap()
    wkT = nc.dram_tensor("wkT", [C, OL], f32, kind="ExternalInput").ap()
    wvT = nc.dram_tensor("wvT", [C, OL], f32, kind="ExternalInput").ap()
    woT = nc.dram_tensor("woT", [OL, C], f32, kind="ExternalInput").ap()
    maskT = None
    if nmix:
        maskT = nc.dram_tensor("maskT", [nmix, 128, 512], f32,
                               kind="ExternalInput").ap()
    yT = nc.dram_tensor("yT", [C, T], f32, kind="ExternalOutput").ap()
    dbg = {}
    if debug:
        dbg["qT"] = nc.dram_tensor("dbg_qT", [NO, 128, T], f32, kind="ExternalOutput").ap()
        dbg["kT"] = nc.dram_tensor("dbg_kT", [NO, 128, T], f32, kind="ExternalOutput").ap()
        dbg["v"] = nc.dram_tensor("dbg_v", [JT, 128, HL * E], f32, kind="ExternalOutput").ap()
        dbg["aT"] = nc.dram_tensor("dbg_aT", [NO, 128, T], f32, kind="ExternalOutput").ap()

    with tile.TileContext(nc) as tc, ExitStack() as ctx:
        # ---- persistent pool (outermost) ----
        persist = ctx.enter_context(tc.tile_pool(name="persist", bufs=1))
        qT_sb = [persist.tile([128, T], f32r, tag=f"qT{o}", name=f"qT{o}") for o in range(NO)]
        kT_sb = [persist.tile([128, T], f32r, tag=f"kT{o}", name=f"kT{o}") for o in range(NO)]
        v_sb = [persist.tile([128, HL * E], f32r, tag=f"v{t}", name=f"v{t}")
                for t in range(JT)]

        # ======== Phases B (V proj) + A (Q/K proj) ========
        with tc.tile_pool(name="pw", bufs=1) as pw, \
             tc.tile_pool(name="px", bufs=2) as px:
            wv_sb = [pw.tile([128, OL], f32r, tag=f"wv{c}", name=f"wv{c}") for c in range(NC)]
            for c in range(NC):
                nc.sync.dma_start(wv_sb[c][:],
                                  wvT[c * 128:(c + 1) * 128, :].bitcast(f32r))
            for t in range(JT):
                nc.vector.memset(v_sb[t][:].bitcast(f32), 1.0)
            wq_sb = [pw.tile([128, OL], f32r, tag=f"wq{c}", name=f"wq{c}") for c in range(NC)]
            wk_sb = [pw.tile([128, OL], f32r, tag=f"wk{c}", name=f"wk{c}") for c in range(NC)]
            for c in range(NC):
                nc.sync.dma_start(wq_sb[c][:],
                                  wqT[c * 128:(c + 1) * 128, :].bitcast(f32r))
                nc.sync.dma_start(wk_sb[c][:],
                                  wkT[c * 128:(c + 1) * 128, :].bitcast(f32r))
            with tc.tile_pool(name="psA", bufs=2, space="PSUM") as psA:
                for tb in range(NTB):
                    xs = [px.tile([128, 512], f32r, tag=f"xs{c}", name=f"xs{c}_{tb}") for c in range(NC)]
                    for c in range(NC):
                        nc.sync.dma_start(
                            xs[c][:],
                            xT[c * 128:(c + 1) * 128, tb * 512:(tb + 1) * 512].bitcast(f32r))
                    psq = [psA.tile([128, 512], f32, tag=f"psq{o}", name=f"psq{o}_{tb}") for o in range(NO)]
                    for c in range(NC):
                        for o in range(NO):
                            nc.tensor.matmul(psq[o][:],
                                             wq_sb[c][:, o * 128:(o + 1) * 128],
                                             xs[c][:], start=(c == 0), stop=(c == NC - 1))
                    for o in range(NO):
                        nc.scalar.copy(qT_sb[o][:, tb * 512:(tb + 1) * 512], psq[o][:])
            # pass 2: K proj ([o,t]) + V proj ([t,o]) sharing the x slices
            with tc.tile_pool(name="psA2", bufs=1, space="PSUM") as psA2:
                for tb in range(NTB):
                    xs = [px.tile([128, 512], f32r, tag=f"xs{c}", name=f"xs2{c}_{tb}") for c in range(NC)]
                    for c in range(NC):
                        nc.sync.dma_start(
                            xs[c][:],
                            xT[c * 128:(c + 1) * 128, tb * 512:(tb + 1) * 512].bitcast(f32r))
                    psk = [psA2.tile([128, 512], f32, tag=f"psk{o}", name=f"psk{o}_{tb}") for o in range(NO)]
                    for c in range(NC):
                        for o in range(NO):
                            nc.tensor.matmul(psk[o][:],
                                             wk_sb[c][:, o * 128:(o + 1) * 128],
                                             xs[c][:], start=(c == 0), stop=(c == NC - 1))
                    for o in range(NO):
                        nc.vector.tensor_copy(kT_sb[o][:, tb * 512:(tb + 1) * 512],
                                              psk[o][:])
                    for sub in range(4):
                        t = tb * 4 + sub
                        psv = psA2.tile([128, OL], f32, tag="psv2", name=f"psv2_{t}", bufs=2)
                        for c in range(NC):
                            nc.tensor.matmul(psv[:], xs[c][:, sub * 128:(sub + 1) * 128],
                                             wv_sb[c][:], start=(c == 0), stop=(c == NC - 1))
                        v3 = v_sb[t][:].rearrange("p (h e) -> p h e", e=E)
                        ps3 = psv[:].rearrange("p (h e) -> p h e", e=D)
                        nc.vector.tensor_copy(v3[:, :, 0:D], ps3[:])

        # aT lives from phase C through phase D
        persist2 = ctx.enter_context(tc.tile_pool(name="persist2", bufs=1))
        aT_sb = [persist2.tile([128, T], f32r, tag=f"aT{o}", name=f"aT{o}") for o in range(NO)]

        # ======== Phase C: attention ========
        with tc.tile_pool(name="pmsk", bufs=1) as pmsk, \
             tc.tile_pool(name="ppp", bufs=4) as ppp, \
             tc.tile_pool(name="pnrm", bufs=2) as pnrm, \
             tc.tile_pool(name="pwo", bufs=1) as pwo, \
             tc.tile_pool(name="pys", bufs=2) as pys, \
             tc.tile_pool(name="psS", bufs=3, space="PSUM") as psS, \
             tc.tile_pool(name="psO", bufs=2, space="PSUM") as psO:
            wo_sb = [pwo.tile([128, C], f32r, tag=f"wo{o}", name=f"wo{o}")
                     for o in range(NO)]
            for o in range(NO):
                nc.sync.dma_start(wo_sb[o][:],
                                  woT[o * 128:(o + 1) * 128, :].bitcast(f32r))
            def emit_oproj(ib_src, ct):
                iis = slice(ib_src * 512, (ib_src + 1) * 512)
                psy = psO.tile([128, 512], f32, tag="po", name=f"psy_{ib_src}_{ct}")
                for o in range(NO):
                    nc.tensor.matmul(psy[:],
                                     wo_sb[o][:, ct * 128:(ct + 1) * 128],
                                     aT_sb[o][:, iis],
                                     start=(o == 0), stop=(o == NO - 1))
                ys = pys.tile([128, 512], f32, tag="ys", name=f"ys_{ib_src}_{ct}")
                nc.vector.tensor_copy(ys[:], psy[:])
                nc.sync.dma_start(yT[ct * 128:(ct + 1) * 128, iis], ys[:])

            pending = []   # (ib, ct) o_proj blocks awaiting emission
            for ib in range(IBN):
                js = [jt for jt in range(JT) if kinds[jt][ib] != 0]
                mx = {jt: mixidx[jt][ib] for jt in js if kinds[jt][ib] == 2}
                msk = {}
                for sl, (jt, m) in enumerate(mx.items()):
                    mt = pmsk.tile([128, 512], f32r, tag=f"msk{sl}_{ib % 2}", name=f"msk{sl}_{ib}")
                    nc.sync.dma_start(mt[:], maskT[m].bitcast(f32r))
                    msk[jt] = mt
                ii = slice(ib * 512, (ib + 1) * 512)
                for p in range(NP):
                    for _ in range(2):
                        if pending:
                            emit_oproj(*pending.pop(0))
                    hA, hB = 2 * p, 2 * p + 1
                    poA = psO.tile([65, 512], f32, tag="po", name=f"poA_{ib}_{p}")
                    poB = psO.tile([65, 512], f32, tag="po", name=f"poB_{ib}_{p}")
                    nj = len(js)
                    for cidx, jt in enumerate(js):
                        jj = slice(jt * 128, (jt + 1) * 128)
                        sAB = psS.tile([128, 1024], f32, tag="sS",
                                       name=f"s_{ib}_{p}_{cidx}")
                        nc.tensor.matmul(sAB[:, 0:512], kT_sb[p][0:64, jj],
                                         qT_sb[p][0:64, ii],
                                         start=True, stop=True,
                                         tile_position=(0, 0))
                        nc.tensor.matmul(sAB[:, 512:1024], kT_sb[p][64:128, jj],
                                         qT_sb[p][64:128, ii],
                                         start=True, stop=True,
                                         tile_position=(64, 0))
                        pAB = ppp.tile([128, 1024], f32r, tag="pP",
                                       name=f"pp_{ib}_{p}_{cidx}")
                        nc.scalar.activation(pAB[:], sAB[:], EXP, scale=SCALE)
                        if jt in msk:
                            nc.vector.tensor_mul(pAB[:, 0:512],
                                                 pAB[:, 0:512].bitcast(f32),
                                                 msk[jt][:].bitcast(f32))
                            nc.vector.tensor_mul(pAB[:, 512:1024],
                                                 pAB[:, 512:1024].bitcast(f32),
                                                 msk[jt][:].bitcast(f32))
                        first = (cidx == 0)
                        last = (cidx == nj - 1)
                        nc.tensor.matmul(poA[:], v_sb[jt][:, hA * E:(hA + 1) * E],
                                         pAB[:, 0:512], start=first, stop=last)
                        nc.tensor.matmul(poB[:], v_sb[jt][:, hB * E:(hB + 1) * E],
                                         pAB[:, 512:1024], start=first, stop=last)
                    dnA = pnrm.tile([1, 512], f32, tag="dnA", name=f"dnA_{ib}_{p}")
                    dnB = pnrm.tile([1, 512], f32, tag="dnB", name=f"dnB_{ib}_{p}")
                    nc.vector.tensor_copy(dnA[:], poA[64:65, :])
                    nc.vector.tensor_copy(dnB[:], poB[64:65, :])
                    rrA = pnrm.tile([1, 512], f32, tag="rrA", name=f"rrA_{ib}_{p}")
                    rrB = pnrm.tile([1, 512], f32, tag="rrB", name=f"rrB_{ib}_{p}")
                    nc.vector.reciprocal_approx_fast(rrA[:], dnA[:])
                    nc.vector.reciprocal_approx_fast(rrB[:], dnB[:])
                    bcA = pnrm.tile([64, 512], f32, tag="bcA", name=f"bcA_{ib}_{p}", bufs=1)
                    bcB = pnrm.tile([64, 512], f32, tag="bcB", name=f"bcB_{ib}_{p}", bufs=1)
                    nc.gpsimd.partition_broadcast(bcA[:], rrA[:])
                    nc.gpsimd.partition_broadcast(bcB[:], rrB[:])
                    nc.vector.tensor_mul(aT_sb[p][0:64, ii], poA[0:64, :], bcA[:])
                    stgB = pnrm.tile([64, 512], f32r, tag="stgB", name=f"stgB_{ib}_{p}")
                    nc.vector.tensor_mul(stgB[:], poB[0:64, :], bcB[:])
                    nc.sync.dma_start(aT_sb[p][64:128, ii], stgB[:])
                pending.extend((ib, ct) for ct in range(NC))
            for ib_src, ct in pending:
                emit_oproj(ib_src, ct)

        if debug:
            for o in range(NO):
                nc.sync.dma_start(dbg["aT"][o], aT_sb[o][:].bitcast(f32))

    nc.compile()
    return nc


def classify_mask(mask2d, T):
    """mask2d: [T, T] (i=query rows, j=key cols). Returns kinds, mixidx, tiles."""
    import numpy as np
    JT, IBN = T // 128, T // 512
    kinds = [[0] * IBN for _ in range(JT)]
    mixidx = [[-1] * IBN for _ in range(JT)]
    tiles = []
    for jt in range(JT):
        for ib in range(IBN):
            blk = mask2d[ib * 512:(ib + 1) * 512, jt * 128:(jt + 1) * 128]
            if not blk.any():
                kinds[jt][ib] = 0
            elif blk.all():
                kinds[jt][ib] = 1
            else:
                kinds[jt][ib] = 2
                mixidx[jt][ib] = len(tiles)
                tiles.append(np.ascontiguousarray(blk.T.astype(np.float32)))
    tiles = np.stack(tiles) if tiles else None
    return kinds, mixidx, tiles


B, T, C = 4, 2048, 1024
H, HD = 16, 64
G = 2
HL = H // G
OL = HL * HD

_cache = {}


def kernel(x, mask, wq, wk, wv, wo):
    from concourse import bass_utils
    x = np.asarray(x, dtype=np.float32)
    mask = np.asarray(mask)
    wq = np.asarray(wq, dtype=np.float32)
    wk = np.asarray(wk, dtype=np.float32)
    wv = np.asarray(wv, dtype=np.float32)
    wo = np.asarray(wo, dtype=np.float32)

    mask2d = mask.reshape(mask.shape[-2], mask.shape[-1])
    kinds, mixidx, tiles = classify_mask(mask2d, T)
    nmix = 0 if tiles is None else len(tiles)

    key = tuple(tuple(r) for r in kinds)
    if key not in _cache:
        _cache[key] = build(T=T, C=C, HL=HL, D=HD, kinds=kinds, mixidx=mixidx,
                            nmix=nmix, n_cores=8)
    nc = _cache[key]

    in_maps = []
    for b in range(B):
        for g in range(G):
            m = {
                "xT": np.ascontiguousarray(x[b].T),
                "wqT": np.ascontiguousarray(wq[g * OL:(g + 1) * OL, :].T),
                "wkT": np.ascontiguousarray(wk[g * OL:(g + 1) * OL, :].T),
                "wvT": np.ascontiguousarray(wv[g * OL:(g + 1) * OL, :].T),
                "woT": np.ascontiguousarray(wo[:, g * OL:(g + 1) * OL].T),
            }
            if nmix:
                m["maskT"] = tiles
            in_maps.append(m)

    res = bass_utils.run_bass_kernel_spmd(nc, in_maps, core_ids=list(range(8)))
    out = np.empty((B, T, C), np.float32)
    for b in range(B):
        out[b] = (res.results[2 * b]["yT"] + res.results[2 * b + 1]["yT"]).T
    return out



# revision 8
# speedup vs baseline: 1.2020x; 1.2020x over previous
"""Self-contained TRN2 Bass kernel for causal multi-head attention.

Problem: x[4,2048,1024], causal mask, wq/wk/wv/wo [1024,1024], H=16, HD=64.
Sharding: 8 NeuronCores = 4 batches x 2 head-groups (8 heads each).
Each core computes Q/K/V projections for its head group, causal attention
(block-skipped via host mask classification), and a partial o_proj; the two
partial outputs per batch are summed on host (the tensor-parallel
all-reduce of the unshard step).

v2 layout (vs v1 baseline):
  - single pass over x for Q/K/V projections (x DMA'd once)
  - persistent deduped causal-mask tiles in SBUF, [128,1024] bf16 with the
    pattern duplicated so one multiply covers both packed heads
  - bf16 for qT/kT/v/pAB/mask/aT/wo (same PE rate as f32r, 2-4x DVE rate,
    half the SBUF/DMA)
  - PSUM plan: score ring 2x[128,1024] (4 banks, also serves o_proj psy) +
    4 single-bank po tiles (two alternating pairs, so softmax-normalize
    never blocks the next head-pair's AV accumulation)
  - o_proj emits psy[128q, 1024c] = sum_o aT_chunk^T @ wo, y stored [T,C]
    so the output DMA is contiguous and the host drops a transpose
"""
import sys
sys.path.insert(0, "/opt/trn_rl_repo")

import numpy as np
import ml_dtypes

from contextlib import ExitStack

import concourse.bass as bass
import concourse.mybir as mybir
import concourse.tile as tile
from concourse import bacc

f32 = mybir.dt.float32
f32r = mybir.dt.float32r
bf16 = mybir.dt.bfloat16
EXP = mybir.ActivationFunctionType.Exp


def build(T=2048, C=1024, HL=8, D=64, kinds=None, mixpat=None, npat=0,
          n_cores=8, debug=False):
    OL = HL * D
    JT = T // 128
    IBN = T // 512
    NC = C // 128
    NO = OL // 128
    NTB = T // 512
    NP = HL // 2
    SCALE = 1.0 / float(D) ** 0.5
    E = D + 1  # v columns per head incl. ones column

    assert kinds is not None

    nc = bacc.Bacc("TRN2", target_bir_lowering=False, debug=False,
                   num_devices=n_cores)

    xT = nc.dram_tensor("xT", [C, T], f32, kind="ExternalInput").ap()
    wqT = nc.dram_tensor("wqT", [C, OL], f32, kind="ExternalInput").ap()
    wkT = nc.dram_tensor("wkT", [C, OL], f32, kind="ExternalInput").ap()
    wvT = nc.dram_tensor("wvT", [C, OL], f32, kind="ExternalInput").ap()
    woT = nc.dram_tensor("woT", [OL, C], bf16, kind="ExternalInput").ap()
    maskT = None
    if npat:
        maskT = nc.dram_tensor("maskT", [npat, 128, 1024], bf16,
                               kind="ExternalInput").ap()
    y = nc.dram_tensor("y", [T, C], f32, kind="ExternalOutput").ap()
    dbg = {}
    if debug:
        NOx = (HL * D) // 128
        dbg["qT"] = nc.dram_tensor("dbg_qT", [NOx, 128, T], bf16, kind="ExternalOutput").ap()
        dbg["kT"] = nc.dram_tensor("dbg_kT", [NOx, 128, T], bf16, kind="ExternalOutput").ap()
        dbg["v"] = nc.dram_tensor("dbg_v", [T // 128, 128, HL * (D + 1)], bf16, kind="ExternalOutput").ap()
        dbg["aT"] = nc.dram_tensor("dbg_aT", [NOx, 128, T], bf16, kind="ExternalOutput").ap()
        dbg["wo"] = nc.dram_tensor("dbg_wo", [NOx, 128, C], bf16, kind="ExternalOutput").ap()
        dbg["msk"] = nc.dram_tensor("dbg_msk", [max(npat, 1), 128, 1024], bf16, kind="ExternalOutput").ap()

    with tile.TileContext(nc) as tc, ExitStack() as ctx:
        # ---- persistent SBUF ----
        persist = ctx.enter_context(tc.tile_pool(name="persist", bufs=1))
        qT_sb = [persist.tile([128, T], bf16, tag=f"qT{o}", name=f"qT{o}")
                 for o in range(NO)]
        kT_sb = [persist.tile([128, T], bf16, tag=f"kT{o}", name=f"kT{o}")
                 for o in range(NO)]
        v_sb = [persist.tile([128, HL * E], bf16, tag=f"v{t}", name=f"v{t}")
                for t in range(JT)]
        wo_sb = [persist.tile([128, C], bf16, tag=f"wo{o}", name=f"wo{o}")
                 for o in range(NO)]
        msk_sb = [persist.tile([128, 1024], bf16, tag=f"msk{s}", name=f"msk{s}")
                  for s in range(npat)]
        aT_sb = [persist.tile([128, T], bf16, tag=f"aT{o}", name=f"aT{o}")
                 for o in range(NO)]

        # ======== Phase A: Q/K/V projections, one pass over x ========
        with tc.tile_pool(name="pw", bufs=1) as pw, \
             tc.tile_pool(name="px", bufs=2) as px, \
             tc.tile_pool(name="psA", bufs=6, space="PSUM") as psA:
            wq_sb = [pw.tile([128, OL], f32r, tag=f"wq{c}", name=f"wq{c}") for c in range(NC)]
            wk_sb = [pw.tile([128, OL], f32r, tag=f"wk{c}", name=f"wk{c}") for c in range(NC)]
            wv_sb = [pw.tile([128, OL], f32r, tag=f"wv{c}", name=f"wv{c}") for c in range(NC)]
            xs0 = [px.tile([128, 512], f32r, tag=f"xs{c}", name=f"xs{c}_t0")
                   for c in range(NC)]
            # first compute needs wq + x(tb0): interleave those DMAs first
            for c in range(NC):
                nc.sync.dma_start(wq_sb[c][:],
                                  wqT[c * 128:(c + 1) * 128, :].bitcast(f32r))
                nc.sync.dma_start(
                    xs0[c][:], xT[c * 128:(c + 1) * 128, 0:512].bitcast(f32r))
            for c in range(NC):
                nc.sync.dma_start(wk_sb[c][:],
                                  wkT[c * 128:(c + 1) * 128, :].bitcast(f32r))
            for c in range(NC):
                nc.sync.dma_start(wv_sb[c][:],
                                  wvT[c * 128:(c + 1) * 128, :].bitcast(f32r))
            for o in range(NO):
                nc.sync.dma_start(wo_sb[o][:], woT[o * 128:(o + 1) * 128, :])
            for s in range(npat):
                nc.sync.dma_start(msk_sb[s][:], maskT[s])
            for t in range(JT):
                nc.vector.memset(v_sb[t][:], 1.0)

            for tb in range(NTB):
                tbs = slice(tb * 512, (tb + 1) * 512)
                if tb == 0:
                    xs = xs0
                else:
                    xs = [px.tile([128, 512], f32r, tag=f"xs{c}",
                                  name=f"xs{c}_t{tb}") for c in range(NC)]
                    for c in range(NC):
                        nc.sync.dma_start(
                            xs[c][:],
                            xT[c * 128:(c + 1) * 128, tbs].bitcast(f32r))
                for o in range(NO):
                    psq = psA.tile([128, 512], f32, tag="pa", name=f"psq{o}_{tb}")
                    for c in range(NC):
                        nc.tensor.matmul(psq[:], wq_sb[c][:, o * 128:(o + 1) * 128],
                                         xs[c][:], start=(c == 0), stop=(c == NC - 1))
                    nc.scalar.copy(qT_sb[o][:, tbs], psq[:])
                for o in range(NO):
                    psk = psA.tile([128, 512], f32, tag="pa", name=f"psk{o}_{tb}")
                    for c in range(NC):
                        nc.tensor.matmul(psk[:], wk_sb[c][:, o * 128:(o + 1) * 128],
                                         xs[c][:], start=(c == 0), stop=(c == NC - 1))
                    nc.vector.tensor_copy(kT_sb[o][:, tbs], psk[:])
                for sub in range(4):
                    t = tb * 4 + sub
                    psv = psA.tile([128, OL], f32, tag="pa", name=f"psv_{t}")
                    for c in range(NC):
                        nc.tensor.matmul(psv[:], xs[c][:, sub * 128:(sub + 1) * 128],
                                         wv_sb[c][:], start=(c == 0), stop=(c == NC - 1))
                    v3 = v_sb[t][:].rearrange("p (h e) -> p h e", e=E)
                    ps3 = psv[:].rearrange("p (h e) -> p h e", e=D)
                    nc.vector.tensor_copy(v3[:, :, 0:D], ps3[:])

        # ======== Phase C: attention + interleaved o_proj ========
        with tc.tile_pool(name="ppp", bufs=4) as ppp, \
             tc.tile_pool(name="pnrm", bufs=2) as pnrm, \
             tc.tile_pool(name="pys", bufs=2) as pys, \
             tc.tile_pool(name="psS", bufs=2, space="PSUM") as psS, \
             tc.tile_pool(name="po", bufs=1, space="PSUM") as po:

            def emit_oproj(qc):
                qq = slice(qc * 128, (qc + 1) * 128)
                psy = psS.tile([128, C], f32, tag="sS", name=f"psy_{qc}")
                for half in range(2):
                    cs = slice(half * 512, (half + 1) * 512)
                    for o in range(NO):
                        nc.tensor.matmul(psy[:, cs], aT_sb[o][:, qq],
                                         wo_sb[o][:, cs],
                                         start=(o == 0), stop=(o == NO - 1))
                ys = pys.tile([128, C], f32, tag="ys", name=f"ys_{qc}")
                nc.vector.tensor_copy(ys[:], psy[:])
                nc.sync.dma_start(y[qq, :], ys[:])

            pending = []   # o_proj 128-query chunks awaiting emission
            for ib in range(IBN):
                ii = slice(ib * 512, (ib + 1) * 512)
                js = [jt for jt in range(JT) if kinds[jt][ib] != 0]
                nj = len(js)
                for p in range(NP):
                    hA, hB = 2 * p, 2 * p + 1
                    poA = po.tile([E, 512], f32, tag=f"poA{p % 2}",
                                  name=f"poA_{ib}_{p}")
                    poB = po.tile([E, 512], f32, tag=f"poB{p % 2}",
                                  name=f"poB_{ib}_{p}")
                    for cidx, jt in enumerate(js):
                        jj = slice(jt * 128, (jt + 1) * 128)
                        sAB = psS.tile([128, 1024], f32, tag="sS",
                                       name=f"s_{ib}_{p}_{cidx}")
                        nc.tensor.matmul(sAB[:, 0:512], kT_sb[p][0:64, jj],
                                         qT_sb[p][0:64, ii],
                                         start=True, stop=True,
                                         tile_position=(0, 0))
                        nc.tensor.matmul(sAB[:, 512:1024], kT_sb[p][64:128, jj],
                                         qT_sb[p][64:128, ii],
                                         start=True, stop=True,
                                         tile_position=(64, 0))
                        pAB = ppp.tile([128, 1024], bf16, tag="pP",
                                       name=f"pp_{ib}_{p}_{cidx}")
                        nc.scalar.activation(pAB[:], sAB[:], EXP, scale=SCALE)
                        if kinds[jt][ib] == 2:
                            nc.vector.tensor_mul(pAB[:], pAB[:],
                                                 msk_sb[mixpat[jt][ib]][:])
                        first = (cidx == 0)
                        last = (cidx == nj - 1)
                        nc.tensor.matmul(poA[:], v_sb[jt][:, hA * E:(hA + 1) * E],
                                         pAB[:, 0:512], start=first, stop=last)
                        nc.tensor.matmul(poB[:], v_sb[jt][:, hB * E:(hB + 1) * E],
                                         pAB[:, 512:1024], start=first, stop=last)
                    # softmax normalization: denominators sit in row 64 (ones col)
                    dnA = pnrm.tile([1, 512], f32, tag="dnA", name=f"dnA_{ib}_{p}")
                    dnB = pnrm.tile([1, 512], f32, tag="dnB", name=f"dnB_{ib}_{p}")
                    nc.vector.tensor_copy(dnA[:], poA[64:65, :])
                    nc.vector.tensor_copy(dnB[:], poB[64:65, :])
                    rrA = pnrm.tile([1, 512], f32, tag="rrA", name=f"rrA_{ib}_{p}")
                    rrB = pnrm.tile([1, 512], f32, tag="rrB", name=f"rrB_{ib}_{p}")
                    nc.vector.reciprocal_approx_fast(rrA[:], dnA[:])
                    nc.vector.reciprocal_approx_fast(rrB[:], dnB[:])
                    bcA = pnrm.tile([64, 512], f32, tag="bcA", name=f"bcA_{ib}_{p}")
                    bcB = pnrm.tile([64, 512], f32, tag="bcB", name=f"bcB_{ib}_{p}")
                    nc.gpsimd.partition_broadcast(bcA[:], rrA[:])
                    nc.gpsimd.partition_broadcast(bcB[:], rrB[:])
                    nc.vector.tensor_mul(aT_sb[p][0:64, ii], poA[0:64, :], bcA[:])
                    stgB = pnrm.tile([64, 512], bf16, tag="stgB", name=f"stgB_{ib}_{p}")
                    nc.vector.tensor_mul(stgB[:], poB[0:64, :], bcB[:])
                    nc.sync.dma_start(aT_sb[p][64:128, ii], stgB[:])
                    # o_proj of the previous ib, one chunk per head-pair slot
                    if pending:
                        emit_oproj(pending.pop(0))
                pending.extend(range(ib * 4, ib * 4 + 4))
            for qc in pending:
                emit_oproj(qc)

        if debug:
            for o in range(NO):
                nc.sync.dma_start(dbg["qT"][o], qT_sb[o][:])
                nc.sync.dma_start(dbg["kT"][o], kT_sb[o][:])
                nc.sync.dma_start(dbg["aT"][o], aT_sb[o][:])
                nc.sync.dma_start(dbg["wo"][o], wo_sb[o][:])
            for t in range(JT):
                nc.sync.dma_start(dbg["v"][t], v_sb[t][:])
            for s in range(npat):
                nc.sync.dma_start(dbg["msk"][s], msk_sb[s][:])

    nc.compile()
    return nc


def classify_mask(mask2d, T):
    """mask2d: [T, T] (i=query rows, j=key cols).

    Returns kinds[jt][ib] in {0 empty, 1 full, 2 mixed}, mixpat[jt][ib]
    (index into the deduped pattern list), and patterns [n, 128, 1024]
    float32 (key-major tiles, duplicated along the free axis so one
    multiply covers both packed heads)."""
    JT, IBN = T // 128, T // 512
    kinds = [[0] * IBN for _ in range(JT)]
    mixpat = [[-1] * IBN for _ in range(JT)]
    patterns = []
    seen = {}
    for jt in range(JT):
        for ib in range(IBN):
            blk = mask2d[ib * 512:(ib + 1) * 512, jt * 128:(jt + 1) * 128]
            if not blk.any():
                kinds[jt][ib] = 0
            elif blk.all():
                kinds[jt][ib] = 1
            else:
                kinds[jt][ib] = 2
                tileT = np.ascontiguousarray(blk.T.astype(np.float32))
                key = tileT.tobytes()
                if key not in seen:
                    seen[key] = len(patterns)
                    patterns.append(np.concatenate([tileT, tileT], axis=1))
                mixpat[jt][ib] = seen[key]
    pat = np.stack(patterns) if patterns else None
    return kinds, mixpat, pat


B, T, C = 4, 2048, 1024
H, HD = 16, 64
G = 2
HL = H // G
OL = HL * HD

_cache = {}


def _prepare(x, mask, wq, wk, wv, wo):
    """Classify the mask, build (or reuse) the compiled kernel, and build
    the 8 per-core input maps."""
    x = np.asarray(x, dtype=np.float32)
    mask = np.asarray(mask)
    wq = np.asarray(wq, dtype=np.float32)
    wk = np.asarray(wk, dtype=np.float32)
    wv = np.asarray(wv, dtype=np.float32)
    wo = np.asarray(wo, dtype=np.float32)

    mask2d = mask.reshape(mask.shape[-2], mask.shape[-1])
    kinds, mixpat, pat = classify_mask(mask2d, T)
    npat = 0 if pat is None else len(pat)
    pat_bf = None if pat is None else pat.astype(ml_dtypes.bfloat16)

    key = tuple(tuple(r) for r in kinds)
    if key not in _cache:
        _cache[key] = build(T=T, C=C, HL=HL, D=HD, kinds=kinds, mixpat=mixpat,
                            npat=npat, n_cores=8)
    nc = _cache[key]

    in_maps = []
    for b in range(B):
        for g in range(G):
            m = {
                "xT": np.ascontiguousarray(x[b].T),
                "wqT": np.ascontiguousarray(wq[g * OL:(g + 1) * OL, :].T),
                "wkT": np.ascontiguousarray(wk[g * OL:(g + 1) * OL, :].T),
                "wvT": np.ascontiguousarray(wv[g * OL:(g + 1) * OL, :].T),
                "woT": np.ascontiguousarray(
                    wo[:, g * OL:(g + 1) * OL].T).astype(ml_dtypes.bfloat16),
            }
            if npat:
                m["maskT"] = pat_bf
            in_maps.append(m)
    return nc, in_maps


def _gather(results):
    out = np.empty((B, T, C), np.float32)
    for b in range(B):
        out[b] = results[2 * b]["y"] + results[2 * b + 1]["y"]
    return out


def kernel(x, mask, wq, wk, wv, wo):
    from concourse import bass_utils
    nc, in_maps = _prepare(x, mask, wq, wk, wv, wo)
    res = bass_utils.run_bass_kernel_spmd(nc, in_maps, core_ids=list(range(8)))
    return _gather(res.results)


# revision 9
# speedup vs baseline: 1.2360x; 1.0282x over previous
"""Self-contained TRN2 Bass kernel for causal multi-head attention.

Problem: x[4,2048,1024], causal mask, wq/wk/wv/wo [1024,1024], H=16, HD=64.
Sharding: 8 NeuronCores = 4 batches x 2 head-groups (8 heads each).
Each core computes Q/K/V projections for its head group, causal attention
(block-skipped via host mask classification), and a partial o_proj; the two
partial outputs per batch are summed on host (the tensor-parallel
all-reduce of the unshard step).

v3: fully fused schedule. The Act engine's exp stream is the second-longest
engine total, so attention starts as early as possible: a short PE-only
front phase does K/V projections for all blocks plus Q(0); the Q(ib+1)
projection and o_proj chunks are interleaved into the attention loop.
Everything is bf16 except PSUM accumulation and the softmax normalization.
One PSUM plan serves the whole kernel: a 2-deep [128,1024] ring (score
tiles, o_proj psy tiles, projection pair-chunks) + 4 single-bank po tiles
(two alternating pairs for the packed-head AV accumulators).
"""
import sys
sys.path.insert(0, "/opt/trn_rl_repo")

import numpy as np
import ml_dtypes

from contextlib import ExitStack

import concourse.bass as bass
import concourse.mybir as mybir
import concourse.tile as tile
from concourse import bacc

f32 = mybir.dt.float32
bf16 = mybir.dt.bfloat16
EXP = mybir.ActivationFunctionType.Exp


def build(T=2048, C=1024, HL=8, D=64, kinds=None, mixpat=None, npat=0,
          n_cores=8, debug=False):
    OL = HL * D
    JT = T // 128
    IBN = T // 512
    NC = C // 128
    NO = OL // 128
    NTB = T // 512
    NP = HL // 2
    SCALE = 1.0 / float(D) ** 0.5
    E = D + 1  # v columns per head incl. ones column

    assert kinds is not None

    nc = bacc.Bacc("TRN2", target_bir_lowering=False, debug=False,
                   num_devices=n_cores)

    xT = nc.dram_tensor("xT", [C, T], bf16, kind="ExternalInput").ap()
    wqT = nc.dram_tensor("wqT", [C, OL], bf16, kind="ExternalInput").ap()
    wkT = nc.dram_tensor("wkT", [C, OL], bf16, kind="ExternalInput").ap()
    wvT = nc.dram_tensor("wvT", [C, OL], bf16, kind="ExternalInput").ap()
    woT = nc.dram_tensor("woT", [OL, C], bf16, kind="ExternalInput").ap()
    maskT = None
    if npat:
        maskT = nc.dram_tensor("maskT", [npat, 128, 1024], bf16,
                               kind="ExternalInput").ap()
    y = nc.dram_tensor("y", [T, C], f32, kind="ExternalOutput").ap()
    dbg = {}
    if debug:
        dbg["qT"] = nc.dram_tensor("dbg_qT", [NO, 128, T], bf16, kind="ExternalOutput").ap()
        dbg["kT"] = nc.dram_tensor("dbg_kT", [NO, 128, T], bf16, kind="ExternalOutput").ap()
        dbg["v"] = nc.dram_tensor("dbg_v", [JT, 128, HL * E], bf16, kind="ExternalOutput").ap()
        dbg["aT"] = nc.dram_tensor("dbg_aT", [NO, 128, T], bf16, kind="ExternalOutput").ap()

    with tile.TileContext(nc) as tc, ExitStack() as ctx:
        # ---- pools live for the whole kernel ----
        persist = ctx.enter_context(tc.tile_pool(name="persist", bufs=1))
        px = ctx.enter_context(tc.tile_pool(name="px", bufs=4))
        ppp = ctx.enter_context(tc.tile_pool(name="ppp", bufs=4))
        pnrm = ctx.enter_context(tc.tile_pool(name="pnrm", bufs=2))
        pys = ctx.enter_context(tc.tile_pool(name="pys", bufs=2))
        psS = ctx.enter_context(tc.tile_pool(name="psS", bufs=2, space="PSUM"))
        po = ctx.enter_context(tc.tile_pool(name="po", bufs=1, space="PSUM"))

        qT_sb = [persist.tile([128, T], bf16, tag=f"qT{o}", name=f"qT{o}")
                 for o in range(NO)]
        kT_sb = [persist.tile([128, T], bf16, tag=f"kT{o}", name=f"kT{o}")
                 for o in range(NO)]
        v_sb = [persist.tile([128, HL * E], bf16, tag=f"v{t}", name=f"v{t}")
                for t in range(JT)]
        wo_sb = [persist.tile([128, C], bf16, tag=f"wo{o}", name=f"wo{o}")
                 for o in range(NO)]
        msk_sb = [persist.tile([128, 1024], bf16, tag=f"msk{s}", name=f"msk{s}")
                  for s in range(npat)]
        aT_sb = [persist.tile([128, T], bf16, tag=f"aT{o}", name=f"aT{o}")
                 for o in range(NO)]
        wq_sb = [persist.tile([128, OL], bf16, tag=f"wq{c}", name=f"wq{c}") for c in range(NC)]
        wk_sb = [persist.tile([128, OL], bf16, tag=f"wk{c}", name=f"wk{c}") for c in range(NC)]
        wv_sb = [persist.tile([128, OL], bf16, tag=f"wv{c}", name=f"wv{c}") for c in range(NC)]

        # xs tiles persist for the whole kernel (bufs=4, one per tb)
        xs_t = [[None] * NC for _ in range(NTB)]

        # first compute needs wk + x(tb0): those DMAs go first
        for c in range(NC):
            nc.sync.dma_start(wk_sb[c][:], wkT[c * 128:(c + 1) * 128, :])
            xs_t[0][c] = px.tile([128, 512], bf16, tag=f"xs{c}", name=f"xs{c}_t0")
            nc.sync.dma_start(xs_t[0][c][:], xT[c * 128:(c + 1) * 128, 0:512])
        for c in range(NC):
            nc.sync.dma_start(wv_sb[c][:], wvT[c * 128:(c + 1) * 128, :])
            nc.sync.dma_start(wq_sb[c][:], wqT[c * 128:(c + 1) * 128, :])
        for tb in range(1, NTB):
            for c in range(NC):
                xs_t[tb][c] = px.tile([128, 512], bf16, tag=f"xs{c}",
                                      name=f"xs{c}_t{tb}")
                nc.sync.dma_start(
                    xs_t[tb][c][:],
                    xT[c * 128:(c + 1) * 128, tb * 512:(tb + 1) * 512])
        for o in range(NO):
            nc.sync.dma_start(wo_sb[o][:], woT[o * 128:(o + 1) * 128, :])
        for s in range(npat):
            nc.sync.dma_start(msk_sb[s][:], maskT[s])
        for t in range(JT):
            nc.vector.memset(v_sb[t][:], 1.0)

        def proj_pair(w_sb, dst, o0, tb, kind):
            """Two [128,512] projection chunks (o0, o0+1) in one ring slot.
            kind 'qk': out [od, t] -> dst[o][:, tb cols]; copies on Act(q)/DVE(k).
            """
            tbs = slice(tb * 512, (tb + 1) * 512)
            ps = psS.tile([128, 1024], f32, tag="sS", name=f"pj_{kind}_{o0}_{tb}")
            for half in range(2):
                o = o0 + half
                hp = ps[:, half * 512:(half + 1) * 512]
                for c in range(NC):
                    nc.tensor.matmul(hp, w_sb[c][:, o * 128:(o + 1) * 128],
                                     xs_t[tb][c][:], start=(c == 0),
                                     stop=(c == NC - 1))
            for half in range(2):
                o = o0 + half
                hp = ps[:, half * 512:(half + 1) * 512]
                if kind == "q":
                    nc.scalar.copy(dst[o][:, tbs], hp)
                else:
                    nc.vector.tensor_copy(dst[o][:, tbs], hp)

        def vproj_pair(sub0, tb):
            """Two V chunks (sub0, sub0+1): psv [128 t, 512 od] halves."""
            ps = psS.tile([128, 1024], f32, tag="sS", name=f"pj_v_{sub0}_{tb}")
            for half in range(2):
                sub = sub0 + half
                hp = ps[:, half * 512:(half + 1) * 512]
                for c in range(NC):
                    nc.tensor.matmul(hp, xs_t[tb][c][:, sub * 128:(sub + 1) * 128],
                                     wv_sb[c][:], start=(c == 0), stop=(c == NC - 1))
            for half in range(2):
                t = tb * 4 + sub0 + half
                hp = ps[:, half * 512:(half + 1) * 512]
                v3 = v_sb[t][:].rearrange("p (h e) -> p h e", e=E)
                ps3 = hp.rearrange("p (h e) -> p h e", e=D)
                nc.vector.tensor_copy(v3[:, :, 0:D], ps3)

        # ======== front: K/V for all tb, then Q(0) ========
        for tb in range(NTB):
            proj_pair(wk_sb, kT_sb, 0, tb, "k")
            proj_pair(wk_sb, kT_sb, 2, tb, "k")
            vproj_pair(0, tb)
            vproj_pair(2, tb)
        proj_pair(wq_sb, qT_sb, 0, 0, "q")
        proj_pair(wq_sb, qT_sb, 2, 0, "q")

        # ======== attention, with Q(ib+1) and o_proj interleaved ========
        def emit_oproj(qc):
            qq = slice(qc * 128, (qc + 1) * 128)
            psy = psS.tile([128, C], f32, tag="sS", name=f"psy_{qc}")
            for half in range(2):
                cs = slice(half * 512, (half + 1) * 512)
                for o in range(NO):
                    nc.tensor.matmul(psy[:, cs], aT_sb[o][:, qq],
                                     wo_sb[o][:, cs],
                                     start=(o == 0), stop=(o == NO - 1))
            ys = pys.tile([128, C], f32, tag="ys", name=f"ys_{qc}")
            nc.vector.tensor_copy(ys[:], psy[:])
            nc.sync.dma_start(y[qq, :], ys[:])

        pending = []   # o_proj 128-query chunks awaiting emission
        for ib in range(IBN):
            ii = slice(ib * 512, (ib + 1) * 512)
            js = [jt for jt in range(JT) if kinds[jt][ib] != 0]
            nj = len(js)
            for p in range(NP):
                if p < 2 and ib + 1 < NTB:
                    proj_pair(wq_sb, qT_sb, 2 * p, ib + 1, "q")
                hA, hB = 2 * p, 2 * p + 1
                poA = po.tile([E, 512], f32, tag=f"poA{p % 2}",
                              name=f"poA_{ib}_{p}")
                poB = po.tile([E, 512], f32, tag=f"poB{p % 2}",
                              name=f"poB_{ib}_{p}")
                for cidx, jt in enumerate(js):
                    jj = slice(jt * 128, (jt + 1) * 128)
                    sAB = psS.tile([128, 1024], f32, tag="sS",
                                   name=f"s_{ib}_{p}_{cidx}")
                    nc.tensor.matmul(sAB[:, 0:512], kT_sb[p][0:64, jj],
                                     qT_sb[p][0:64, ii],
                                     start=True, stop=True,
                                     tile_position=(0, 0))
                    nc.tensor.matmul(sAB[:, 512:1024], kT_sb[p][64:128, jj],
                                     qT_sb[p][64:128, ii],
                                     start=True, stop=True,
                                     tile_position=(64, 0))
                    pAB = ppp.tile([128, 1024], bf16, tag="pP",
                                   name=f"pp_{ib}_{p}_{cidx}")
                    nc.scalar.activation(pAB[:], sAB[:], EXP, scale=SCALE)
                    if kinds[jt][ib] == 2:
                        nc.vector.tensor_mul(pAB[:], pAB[:],
                                             msk_sb[mixpat[jt][ib]][:])
                    first = (cidx == 0)
                    last = (cidx == nj - 1)
                    nc.tensor.matmul(poA[:], v_sb[jt][:, hA * E:(hA + 1) * E],
                                     pAB[:, 0:512], start=first, stop=last)
                    nc.tensor.matmul(poB[:], v_sb[jt][:, hB * E:(hB + 1) * E],
                                     pAB[:, 512:1024], start=first, stop=last)
                # softmax normalization: denominators sit in row 64 (ones col)
                dnA = pnrm.tile([1, 512], f32, tag="dnA", name=f"dnA_{ib}_{p}")
                dnB = pnrm.tile([1, 512], f32, tag="dnB", name=f"dnB_{ib}_{p}")
                nc.vector.tensor_copy(dnA[:], poA[64:65, :])
                nc.vector.tensor_copy(dnB[:], poB[64:65, :])
                rrA = pnrm.tile([1, 512], f32, tag="rrA", name=f"rrA_{ib}_{p}")
                rrB = pnrm.tile([1, 512], f32, tag="rrB", name=f"rrB_{ib}_{p}")
                nc.vector.reciprocal_approx_fast(rrA[:], dnA[:])
                nc.vector.reciprocal_approx_fast(rrB[:], dnB[:])
                bcA = pnrm.tile([64, 512], f32, tag="bcA", name=f"bcA_{ib}_{p}")
                bcB = pnrm.tile([64, 512], f32, tag="bcB", name=f"bcB_{ib}_{p}")
                nc.gpsimd.partition_broadcast(bcA[:], rrA[:])
                nc.gpsimd.partition_broadcast(bcB[:], rrB[:])
                nc.vector.tensor_mul(aT_sb[p][0:64, ii], poA[0:64, :], bcA[:])
                stgB = pnrm.tile([64, 512], bf16, tag="stgB", name=f"stgB_{ib}_{p}")
                nc.vector.tensor_mul(stgB[:], poB[0:64, :], bcB[:])
                nc.sync.dma_start(aT_sb[p][64:128, ii], stgB[:])
                if pending:
                    emit_oproj(pending.pop(0))
            pending.extend(range(ib * 4, ib * 4 + 4))
        for qc in pending:
            emit_oproj(qc)

        if debug:
            for o in range(NO):
                nc.sync.dma_start(dbg["qT"][o], qT_sb[o][:])
                nc.sync.dma_start(dbg["kT"][o], kT_sb[o][:])
                nc.sync.dma_start(dbg["aT"][o], aT_sb[o][:])
            for t in range(JT):
                nc.sync.dma_start(dbg["v"][t], v_sb[t][:])

    nc.compile()
    return nc


def classify_mask(mask2d, T):
    """mask2d: [T, T] (i=query rows, j=key cols).

    Returns kinds[jt][ib] in {0 empty, 1 full, 2 mixed}, mixpat[jt][ib]
    (index into the deduped pattern list), and patterns [n, 128, 1024]
    float32 (key-major tiles, duplicated along the free axis so one
    multiply covers both packed heads)."""
    JT, IBN = T // 128, T // 512
    kinds = [[0] * IBN for _ in range(JT)]
    mixpat = [[-1] * IBN for _ in range(JT)]
    patterns = []
    seen = {}
    for jt in range(JT):
        for ib in range(IBN):
            blk = mask2d[ib * 512:(ib + 1) * 512, jt * 128:(jt + 1) * 128]
            if not blk.any():
                kinds[jt][ib] = 0
            elif blk.all():
                kinds[jt][ib] = 1
            else:
                kinds[jt][ib] = 2
                tileT = np.ascontiguousarray(blk.T.astype(np.float32))
                key = tileT.tobytes()
                if key not in seen:
                    seen[key] = len(patterns)
                    patterns.append(np.concatenate([tileT, tileT], axis=1))
                mixpat[jt][ib] = seen[key]
    pat = np.stack(patterns) if patterns else None
    return kinds, mixpat, pat


B, T, C = 4, 2048, 1024
H, HD = 16, 64
G = 2
HL = H // G
OL = HL * HD

_cache = {}


def _prepare(x, mask, wq, wk, wv, wo):
    """Classify the mask, build (or reuse) the compiled kernel, and build
    the 8 per-core input maps."""
    bf = ml_dtypes.bfloat16
    x = np.asarray(x, dtype=np.float32)
    mask = np.asarray(mask)
    wq = np.asarray(wq, dtype=np.float32)
    wk = np.asarray(wk, dtype=np.float32)
    wv = np.asarray(wv, dtype=np.float32)
    wo = np.asarray(wo, dtype=np.float32)

    mask2d = mask.reshape(mask.shape[-2], mask.shape[-1])
    kinds, mixpat, pat = classify_mask(mask2d, T)
    npat = 0 if pat is None else len(pat)
    pat_bf = None if pat is None else pat.astype(bf)

    key = tuple(tuple(r) for r in kinds)
    if key not in _cache:
        _cache[key] = build(T=T, C=C, HL=HL, D=HD, kinds=kinds, mixpat=mixpat,
                            npat=npat, n_cores=8)
    nc = _cache[key]

    in_maps = []
    for b in range(B):
        for g in range(G):
            m = {
                "xT": np.ascontiguousarray(x[b].T).astype(bf),
                "wqT": np.ascontiguousarray(wq[g * OL:(g + 1) * OL, :].T).astype(bf),
                "wkT": np.ascontiguousarray(wk[g * OL:(g + 1) * OL, :].T).astype(bf),
                "wvT": np.ascontiguousarray(wv[g * OL:(g + 1) * OL, :].T).astype(bf),
                "woT": np.ascontiguousarray(wo[:, g * OL:(g + 1) * OL].T).astype(bf),
            }
            if npat:
                m["maskT"] = pat_bf
            in_maps.append(m)
    return nc, in_maps


def _gather(results):
    out = np.empty((B, T, C), np.float32)
    for b in range(B):
        out[b] = results[2 * b]["y"] + results[2 * b + 1]["y"]
    return out


def kernel(x, mask, wq, wk, wv, wo):
    from concourse import bass_utils
    nc, in_maps = _prepare(x, mask, wq, wk, wv, wo)
    res = bass_utils.run_bass_kernel_spmd(nc, in_maps, core_ids=list(range(8)))
    return _gather(res.results)


# revision 14
# speedup vs baseline: 1.3438x; 1.0872x over previous
"""Self-contained TRN2 Bass kernel for causal multi-head attention.

Problem: x[4,2048,1024], causal mask, wq/wk/wv/wo [1024,1024], H=16, HD=64.
Sharding: 8 NeuronCores = 4 batches x 2 head-groups (8 heads each).
Each core computes Q/K/V projections for its head group, causal attention
(block-skipped via host mask classification), and a partial o_proj; the two
partial outputs per batch are summed on host (the tensor-parallel
all-reduce of the unshard step).

v3: fully fused schedule. The Act engine's exp stream is the second-longest
engine total, so attention starts as early as possible: a short PE-only
front phase does K/V projections for all blocks plus Q(0); the Q(ib+1)
projection and o_proj chunks are interleaved into the attention loop.
Everything is bf16 except PSUM accumulation and the softmax normalization.
One PSUM plan serves the whole kernel: a 2-deep [128,1024] ring (score
tiles, o_proj psy tiles, projection pair-chunks) + 4 single-bank po tiles
(two alternating pairs for the packed-head AV accumulators).
"""
import sys
sys.path.insert(0, "/opt/trn_rl_repo")

import numpy as np
import ml_dtypes

from contextlib import ExitStack

import concourse.bass as bass
import concourse.mybir as mybir
import concourse.tile as tile
from concourse import bacc

f32 = mybir.dt.float32
bf16 = mybir.dt.bfloat16
EXP = mybir.ActivationFunctionType.Exp


def build(T=2048, C=1024, HL=8, D=64, kinds=None, mixpat=None, mixtrim=None,
          npat=0, n_cores=8, debug=False):
    OL = HL * D
    JT = T // 128
    IBN = T // 512
    NC = C // 128
    NO = OL // 128
    NTB = T // 512
    NP = HL // 2
    SCALE = 1.0 / float(D) ** 0.5
    E = D + 1  # v columns per head incl. ones column

    assert kinds is not None

    nc = bacc.Bacc("TRN2", target_bir_lowering=False, debug=False,
                   num_devices=n_cores)

    xT = nc.dram_tensor("xT", [C, T], bf16, kind="ExternalInput").ap()
    wqT = nc.dram_tensor("wqT", [C, OL], bf16, kind="ExternalInput").ap()
    wkT = nc.dram_tensor("wkT", [C, OL], bf16, kind="ExternalInput").ap()
    wvT = nc.dram_tensor("wvT", [C, OL], bf16, kind="ExternalInput").ap()
    woT = nc.dram_tensor("woT", [OL, C], bf16, kind="ExternalInput").ap()
    maskT = None
    if npat:
        maskT = nc.dram_tensor("maskT", [npat, 128, 1024], bf16,
                               kind="ExternalInput").ap()
    y = nc.dram_tensor("y", [T, C], f32, kind="ExternalOutput").ap()
    dbg = {}
    if debug:
        dbg["qT"] = nc.dram_tensor("dbg_qT", [NO, 128, T], bf16, kind="ExternalOutput").ap()
        dbg["kT"] = nc.dram_tensor("dbg_kT", [NO, 128, T], bf16, kind="ExternalOutput").ap()
        dbg["v"] = nc.dram_tensor("dbg_v", [JT, 128, HL * E], bf16, kind="ExternalOutput").ap()
        dbg["aT"] = nc.dram_tensor("dbg_aT", [NO, 128, T], bf16, kind="ExternalOutput").ap()

    with tile.TileContext(nc) as tc, ExitStack() as ctx:
        # ---- pools live for the whole kernel ----
        persist = ctx.enter_context(tc.tile_pool(name="persist", bufs=1))
        px = ctx.enter_context(tc.tile_pool(name="px", bufs=4))
        ppp = ctx.enter_context(tc.tile_pool(name="ppp", bufs=4))
        pnrm = ctx.enter_context(tc.tile_pool(name="pnrm", bufs=2))
        pys = ctx.enter_context(tc.tile_pool(name="pys", bufs=2))
        psS = ctx.enter_context(tc.tile_pool(name="psS", bufs=2, space="PSUM"))
        po = ctx.enter_context(tc.tile_pool(name="po", bufs=1, space="PSUM"))

        qT_sb = [persist.tile([128, T], bf16, tag=f"qT{o}", name=f"qT{o}")
                 for o in range(NO)]
        kT_sb = [persist.tile([128, T], bf16, tag=f"kT{o}", name=f"kT{o}")
                 for o in range(NO)]
        v_sb = [persist.tile([128, HL * E], bf16, tag=f"v{t}", name=f"v{t}")
                for t in range(JT)]
        wo_sb = [persist.tile([128, C], bf16, tag=f"wo{o}", name=f"wo{o}")
                 for o in range(NO)]
        msk_sb = [persist.tile([128, 1024], bf16, tag=f"msk{s}", name=f"msk{s}")
                  for s in range(npat)]
        aT_sb = [persist.tile([128, T], bf16, tag=f"aT{o}", name=f"aT{o}")
                 for o in range(NO)]
        wq_sb = [persist.tile([128, OL], bf16, tag=f"wq{c}", name=f"wq{c}") for c in range(NC)]
        wk_sb = [persist.tile([128, OL], bf16, tag=f"wk{c}", name=f"wk{c}") for c in range(NC)]
        wv_sb = [persist.tile([128, OL], bf16, tag=f"wv{c}", name=f"wv{c}") for c in range(NC)]

        # xs tiles persist for the whole kernel (bufs=4, one per tb)
        xs_t = [[None] * NC for _ in range(NTB)]

        # first compute needs wk + x(tb0): those DMAs go first
        for c in range(NC):
            nc.sync.dma_start(wk_sb[c][:], wkT[c * 128:(c + 1) * 128, :])
            xs_t[0][c] = px.tile([128, 512], bf16, tag=f"xs{c}", name=f"xs{c}_t0")
            nc.sync.dma_start(xs_t[0][c][:], xT[c * 128:(c + 1) * 128, 0:512])
        for c in range(NC):
            nc.sync.dma_start(wv_sb[c][:], wvT[c * 128:(c + 1) * 128, :])
            nc.sync.dma_start(wq_sb[c][:], wqT[c * 128:(c + 1) * 128, :])
        for tb in range(1, NTB):
            for c in range(NC):
                xs_t[tb][c] = px.tile([128, 512], bf16, tag=f"xs{c}",
                                      name=f"xs{c}_t{tb}")
                nc.sync.dma_start(
                    xs_t[tb][c][:],
                    xT[c * 128:(c + 1) * 128, tb * 512:(tb + 1) * 512])
        for o in range(NO):
            nc.sync.dma_start(wo_sb[o][:], woT[o * 128:(o + 1) * 128, :])
        for s in range(npat):
            nc.sync.dma_start(msk_sb[s][:], maskT[s])
        for t in range(JT):
            nc.vector.memset(v_sb[t][:], 1.0)

        def proj_pair(w_sb, dst, o0, tb, kind):
            """Two [128,512] projection chunks (o0, o0+1) in one ring slot.
            kind 'qk': out [od, t] -> dst[o][:, tb cols]; copies on Act(q)/DVE(k).
            """
            tbs = slice(tb * 512, (tb + 1) * 512)
            ps = psS.tile([128, 1024], f32, tag="sS", name=f"pj_{kind}_{o0}_{tb}")
            for half in range(2):
                o = o0 + half
                hp = ps[:, half * 512:(half + 1) * 512]
                for c in range(NC):
                    nc.tensor.matmul(hp, w_sb[c][:, o * 128:(o + 1) * 128],
                                     xs_t[tb][c][:], start=(c == 0),
                                     stop=(c == NC - 1))
            for half in range(2):
                o = o0 + half
                hp = ps[:, half * 512:(half + 1) * 512]
                if kind == "q":
                    nc.scalar.copy(dst[o][:, tbs], hp)
                else:
                    nc.vector.tensor_copy(dst[o][:, tbs], hp)

        def vproj_pair(sub0, tb):
            """Two V chunks (sub0, sub0+1): psv [128 t, 512 od] halves."""
            ps = psS.tile([128, 1024], f32, tag="sS", name=f"pj_v_{sub0}_{tb}")
            for half in range(2):
                sub = sub0 + half
                hp = ps[:, half * 512:(half + 1) * 512]
                for c in range(NC):
                    nc.tensor.matmul(hp, xs_t[tb][c][:, sub * 128:(sub + 1) * 128],
                                     wv_sb[c][:], start=(c == 0), stop=(c == NC - 1))
            for half in range(2):
                t = tb * 4 + sub0 + half
                hp = ps[:, half * 512:(half + 1) * 512]
                v3 = v_sb[t][:].rearrange("p (h e) -> p h e", e=E)
                ps3 = hp.rearrange("p (h e) -> p h e", e=D)
                nc.vector.tensor_copy(v3[:, :, 0:D], ps3)

        # ======== front: K/V for all tb, then Q(0) ========
        for tb in range(NTB):
            proj_pair(wk_sb, kT_sb, 0, tb, "k")
            proj_pair(wk_sb, kT_sb, 2, tb, "k")
            vproj_pair(0, tb)
            vproj_pair(2, tb)
        proj_pair(wq_sb, qT_sb, 0, 0, "q")
        proj_pair(wq_sb, qT_sb, 2, 0, "q")

        # ======== attention, with Q(ib+1) and o_proj interleaved ========
        def emit_oproj(qc):
            qq = slice(qc * 128, (qc + 1) * 128)
            psy = psS.tile([128, C], f32, tag="sS", name=f"psy_{qc}")
            for half in range(2):
                cs = slice(half * 512, (half + 1) * 512)
                for o in range(NO):
                    nc.tensor.matmul(psy[:, cs], aT_sb[o][:, qq],
                                     wo_sb[o][:, cs],
                                     start=(o == 0), stop=(o == NO - 1))
            ys = pys.tile([128, C], f32, tag="ys", name=f"ys_{qc}")
            nc.vector.tensor_copy(ys[:], psy[:])
            nc.sync.dma_start(y[qq, :], ys[:])

        pending = []   # o_proj 128-query chunks awaiting emission
        for ib in range(IBN):
            ii = slice(ib * 512, (ib + 1) * 512)
            js = [jt for jt in range(JT) if kinds[jt][ib] != 0]
            nj = len(js)
            # fillers: independent PE work inserted between the first QKs and
            # the first AVs of each head-pair so exp latency never stalls PE
            fillers = []
            if pending:
                chunks = list(pending)
                pending.clear()
                fillers.extend(("psy", qc) for qc in chunks)
            if ib + 1 < NTB:
                fillers.extend(("qproj", 2 * pp) for pp in range(2))
            fq = {pi: [] for pi in range(NP)}
            for idx, fl in enumerate(fillers):
                fq[idx % NP].append(fl)

            def run_filler(fl):
                if fl[0] == "psy":
                    emit_oproj(fl[1])
                else:
                    proj_pair(wq_sb, qT_sb, fl[1], ib + 1, "q")

            for p in range(NP):
                hA, hB = 2 * p, 2 * p + 1
                poA = po.tile([E, 512], f32, tag=f"poA{p % 2}",
                              name=f"poA_{ib}_{p}")
                poB = po.tile([E, 512], f32, tag=f"poB{p % 2}",
                              name=f"poB_{ib}_{p}")

                trims = []
                for jt in js:
                    tr = 0
                    if kinds[jt][ib] == 2:
                        tr = mixtrim[jt][ib]
                    trims.append(tr)
                trims[0] = 0  # first block must cover the full accumulator

                def emit_qk(cidx):
                    jt = js[cidx]
                    tr = trims[cidx]
                    jj = slice(jt * 128, (jt + 1) * 128)
                    qq = slice(ib * 512 + tr, (ib + 1) * 512)
                    sAB = psS.tile([128, 1024], f32, tag="sS",
                                   name=f"s_{ib}_{p}_{cidx}")
                    nc.tensor.matmul(sAB[:, tr:512], kT_sb[p][0:64, jj],
                                     qT_sb[p][0:64, qq],
                                     start=True, stop=True,
                                     tile_position=(0, 0))
                    nc.tensor.matmul(sAB[:, 512 + tr:1024], kT_sb[p][64:128, jj],
                                     qT_sb[p][64:128, qq],
                                     start=True, stop=True,
                                     tile_position=(64, 0))
                    pAB = ppp.tile([128, 1024], bf16, tag="pP",
                                   name=f"pp_{ib}_{p}_{cidx}")
                    if tr:
                        s3 = sAB.rearrange("p (h q) -> p h q", q=512)
                        p3 = pAB[:].rearrange("p (h q) -> p h q", q=512)
                        nc.scalar.activation(p3[:, :, tr:512], s3[:, :, tr:512],
                                             EXP, scale=SCALE)
                    else:
                        nc.scalar.activation(pAB[:], sAB[:], EXP, scale=SCALE)
                    if kinds[js[cidx]][ib] == 2:
                        p3 = pAB[:].rearrange("p (h q) -> p h q", q=512)
                        m3 = msk_sb[mixpat[js[cidx]][ib]][:].rearrange(
                            "p (h q) -> p h q", q=512)
                        nc.vector.tensor_mul(p3[:, :, tr:512], p3[:, :, tr:512],
                                             m3[:, :, tr:512])
                    return pAB

                def emit_av(cidx, pAB):
                    jt = js[cidx]
                    tr = trims[cidx]
                    first = (cidx == 0)
                    last = (cidx == nj - 1)
                    nc.tensor.matmul(poA[:, tr:512],
                                     v_sb[jt][:, hA * E:(hA + 1) * E],
                                     pAB[:, tr:512], start=first, stop=last,
                                     skip_group_check=True)
                    nc.tensor.matmul(poB[:, tr:512],
                                     v_sb[jt][:, hB * E:(hB + 1) * E],
                                     pAB[:, 512 + tr:1024], start=first,
                                     stop=last,
                                     skip_group_check=True)

                # software pipeline: QK0, QK1, filler, AV0, QK2, AV1, ...
                pabs = {}
                pabs[0] = emit_qk(0)
                if nj > 1:
                    pabs[1] = emit_qk(1)
                for fl in fq[p]:
                    run_filler(fl)
                for cidx in range(nj):
                    if cidx + 2 < nj:
                        pabs[cidx + 2] = emit_qk(cidx + 2)
                    emit_av(cidx, pabs.pop(cidx))
                # softmax normalization: denominators sit in row 64 (ones col)
                dnA = pnrm.tile([1, 512], f32, tag="dnA", name=f"dnA_{ib}_{p}")
                dnB = pnrm.tile([1, 512], f32, tag="dnB", name=f"dnB_{ib}_{p}")
                nc.vector.tensor_copy(dnA[:], poA[64:65, :])
                nc.vector.tensor_copy(dnB[:], poB[64:65, :])
                rrA = pnrm.tile([1, 512], f32, tag="rrA", name=f"rrA_{ib}_{p}")
                rrB = pnrm.tile([1, 512], f32, tag="rrB", name=f"rrB_{ib}_{p}")
                nc.vector.reciprocal_approx_fast(rrA[:], dnA[:])
                nc.vector.reciprocal_approx_fast(rrB[:], dnB[:])
                bcA = pnrm.tile([64, 512], f32, tag="bcA", name=f"bcA_{ib}_{p}")
                bcB = pnrm.tile([64, 512], f32, tag="bcB", name=f"bcB_{ib}_{p}")
                nc.gpsimd.partition_broadcast(bcA[:], rrA[:])
                nc.gpsimd.partition_broadcast(bcB[:], rrB[:])
                nc.vector.tensor_mul(aT_sb[p][0:64, ii], poA[0:64, :], bcA[:])
                stgB = pnrm.tile([64, 512], bf16, tag="stgB", name=f"stgB_{ib}_{p}")
                nc.vector.tensor_mul(stgB[:], poB[0:64, :], bcB[:])
                nc.sync.dma_start(aT_sb[p][64:128, ii], stgB[:])
                if pending:
                    emit_oproj(pending.pop(0))
            pending.extend(range(ib * 4, ib * 4 + 4))
        for qc in pending:
            emit_oproj(qc)

        if debug:
            for o in range(NO):
                nc.sync.dma_start(dbg["qT"][o], qT_sb[o][:])
                nc.sync.dma_start(dbg["kT"][o], kT_sb[o][:])
                nc.sync.dma_start(dbg["aT"][o], aT_sb[o][:])
            for t in range(JT):
                nc.sync.dma_start(dbg["v"][t], v_sb[t][:])

    nc.compile()
    return nc


def classify_mask(mask2d, T):
    """mask2d: [T, T] (i=query rows, j=key cols).

    Returns kinds[jt][ib] in {0 empty, 1 full, 2 mixed}, mixpat[jt][ib]
    (index into the deduped pattern list), mixtrim[jt][ib] (count of leading
    query columns that are entirely masked, so QK/exp/AV can skip them), and
    patterns [n, 128, 1024] float32 (key-major tiles, duplicated along the
    free axis so one multiply covers both packed heads)."""
    JT, IBN = T // 128, T // 512
    kinds = [[0] * IBN for _ in range(JT)]
    mixpat = [[-1] * IBN for _ in range(JT)]
    mixtrim = [[0] * IBN for _ in range(JT)]
    patterns = []
    seen = {}
    for jt in range(JT):
        for ib in range(IBN):
            blk = mask2d[ib * 512:(ib + 1) * 512, jt * 128:(jt + 1) * 128]
            if not blk.any():
                kinds[jt][ib] = 0
            elif blk.all():
                kinds[jt][ib] = 1
            else:
                kinds[jt][ib] = 2
                tileT = np.ascontiguousarray(blk.T.astype(np.float32))
                key = tileT.tobytes()
                if key not in seen:
                    seen[key] = len(patterns)
                    patterns.append(np.concatenate([tileT, tileT], axis=1))
                mixpat[jt][ib] = seen[key]
                colvalid = tileT.any(axis=0)
                mixtrim[jt][ib] = int(np.argmax(colvalid))
    pat = np.stack(patterns) if patterns else None
    return kinds, mixpat, mixtrim, pat


B, T, C = 4, 2048, 1024
H, HD = 16, 64
G = 2
HL = H // G
OL = HL * HD

_cache = {}


def _prepare(x, mask, wq, wk, wv, wo):
    """Classify the mask, build (or reuse) the compiled kernel, and build
    the 8 per-core input maps."""
    bf = ml_dtypes.bfloat16
    x = np.asarray(x, dtype=np.float32)
    mask = np.asarray(mask)
    wq = np.asarray(wq, dtype=np.float32)
    wk = np.asarray(wk, dtype=np.float32)
    wv = np.asarray(wv, dtype=np.float32)
    wo = np.asarray(wo, dtype=np.float32)

    mask2d = mask.reshape(mask.shape[-2], mask.shape[-1])
    kinds, mixpat, mixtrim, pat = classify_mask(mask2d, T)
    npat = 0 if pat is None else len(pat)
    pat_bf = None if pat is None else pat.astype(bf)

    key = (tuple(tuple(r) for r in kinds), tuple(tuple(r) for r in mixpat),
           tuple(tuple(r) for r in mixtrim))
    if key not in _cache:
        _cache[key] = build(T=T, C=C, HL=HL, D=HD, kinds=kinds, mixpat=mixpat,
                            mixtrim=mixtrim, npat=npat, n_cores=8)
    nc = _cache[key]

    in_maps = []
    for b in range(B):
        for g in range(G):
            m = {
                "xT": np.ascontiguousarray(x[b].T).astype(bf),
                "wqT": np.ascontiguousarray(wq[g * OL:(g + 1) * OL, :].T).astype(bf),
                "wkT": np.ascontiguousarray(wk[g * OL:(g + 1) * OL, :].T).astype(bf),
                "wvT": np.ascontiguousarray(wv[g * OL:(g + 1) * OL, :].T).astype(bf),
                "woT": np.ascontiguousarray(wo[:, g * OL:(g + 1) * OL].T).astype(bf),
            }
            if npat:
                m["maskT"] = pat_bf
            in_maps.append(m)
    return nc, in_maps


def _gather(results):
    out = np.empty((B, T, C), np.float32)
    for b in range(B):
        out[b] = results[2 * b]["y"] + results[2 * b + 1]["y"]
    return out


def kernel(x, mask, wq, wk, wv, wo):
    from concourse import bass_utils
    nc, in_maps = _prepare(x, mask, wq, wk, wv, wo)
    res = bass_utils.run_bass_kernel_spmd(nc, in_maps, core_ids=list(range(8)))
    return _gather(res.results)


# revision 15
# speedup vs baseline: 1.3677x; 1.0178x over previous
"""Self-contained TRN2 Bass kernel for causal multi-head attention.

Problem: x[4,2048,1024], causal mask, wq/wk/wv/wo [1024,1024], H=16, HD=64.
Sharding: 8 NeuronCores = 4 batches x 2 head-groups (8 heads each).
Each core computes Q/K/V projections for its head group, causal attention
(block-skipped via host mask classification), and a partial o_proj; the two
partial outputs per batch are summed on host (the tensor-parallel
all-reduce of the unshard step).

v5: per-block fused schedule. Only Q/K/V of token-block 0 run up front
(~20us); the projections of block tb+1 and the o_proj of block ib-1 are
"fillers" interleaved into attention(ib)'s head-pair loops, keeping the PE
busy through every exp-latency window while the Act engine streams the
softmax exps nearly end-to-end. Everything is bf16 except PSUM accumulation
and the softmax normalization. One PSUM plan serves the whole kernel:
a 2-deep [128,1024] ring (score tiles, o_proj psy tiles, projection
pair-chunks) + 4 single-bank po tiles (two alternating pairs of packed-head
AV accumulators). Inputs are packed so each weight/x-block/mask set is one
DMA. Mixed causal blocks skip their fully-masked leading query columns in
QK/exp/mask/AV (diagonal trimming).
"""
import sys
sys.path.insert(0, "/opt/trn_rl_repo")

import numpy as np
import ml_dtypes

from contextlib import ExitStack

import concourse.bass as bass
import concourse.mybir as mybir
import concourse.tile as tile
from concourse import bacc

f32 = mybir.dt.float32
bf16 = mybir.dt.bfloat16
EXP = mybir.ActivationFunctionType.Exp


def build(T=2048, C=1024, HL=8, D=64, kinds=None, mixpat=None, mixtrim=None,
          npat=0, n_cores=8, debug=False):
    OL = HL * D
    JT = T // 128
    IBN = T // 512
    NC = C // 128
    NO = OL // 128
    NTB = T // 512
    NP = HL // 2
    SCALE = 1.0 / float(D) ** 0.5
    E = D + 1  # v columns per head incl. ones column

    assert kinds is not None

    nc = bacc.Bacc("TRN2", target_bir_lowering=False, debug=False,
                   num_devices=n_cores)

    xT = nc.dram_tensor("xT", [C, T], bf16, kind="ExternalInput").ap()
    wqT = nc.dram_tensor("wqT", [C, OL], bf16, kind="ExternalInput").ap()
    wkT = nc.dram_tensor("wkT", [C, OL], bf16, kind="ExternalInput").ap()
    wvT = nc.dram_tensor("wvT", [C, OL], bf16, kind="ExternalInput").ap()
    woT = nc.dram_tensor("woT", [OL, C], bf16, kind="ExternalInput").ap()
    maskT = None
    if npat:
        maskT = nc.dram_tensor("maskT", [npat, 128, 1024], bf16,
                               kind="ExternalInput").ap()
    y = nc.dram_tensor("y", [T, C], f32, kind="ExternalOutput").ap()
    dbg = {}
    if debug:
        dbg["qT"] = nc.dram_tensor("dbg_qT", [NO, 128, T], bf16, kind="ExternalOutput").ap()
        dbg["kT"] = nc.dram_tensor("dbg_kT", [NO, 128, T], bf16, kind="ExternalOutput").ap()
        dbg["v"] = nc.dram_tensor("dbg_v", [JT, 128, HL * E], bf16, kind="ExternalOutput").ap()
        dbg["aT"] = nc.dram_tensor("dbg_aT", [NO, 128, T], bf16, kind="ExternalOutput").ap()

    with tile.TileContext(nc) as tc, ExitStack() as ctx:
        # ---- pools live for the whole kernel ----
        persist = ctx.enter_context(tc.tile_pool(name="persist", bufs=1))
        ppp = ctx.enter_context(tc.tile_pool(name="ppp", bufs=4))
        pnrm = ctx.enter_context(tc.tile_pool(name="pnrm", bufs=2))
        pys = ctx.enter_context(tc.tile_pool(name="pys", bufs=2))
        psS = ctx.enter_context(tc.tile_pool(name="psS", bufs=2, space="PSUM"))
        po = ctx.enter_context(tc.tile_pool(name="po", bufs=1, space="PSUM"))

        qT_sb = [persist.tile([128, T], bf16, tag=f"qT{o}", name=f"qT{o}")
                 for o in range(NO)]
        kT_sb = [persist.tile([128, T], bf16, tag=f"kT{o}", name=f"kT{o}")
                 for o in range(NO)]
        v_sb = [persist.tile([128, HL * E], bf16, tag=f"v{t}", name=f"v{t}")
                for t in range(JT)]
        aT_sb = [persist.tile([128, T], bf16, tag=f"aT{o}", name=f"aT{o}")
                 for o in range(NO)]
        # packed: one DMA per tensor
        wq_all = persist.tile([128, NC, OL], bf16, tag="wq", name="wq_all")
        wk_all = persist.tile([128, NC, OL], bf16, tag="wk", name="wk_all")
        wv_all = persist.tile([128, NC, OL], bf16, tag="wv", name="wv_all")
        wo_all = persist.tile([128, NO, C], bf16, tag="wo", name="wo_all")
        msk_all = None
        if npat:
            msk_all = persist.tile([128, npat, 1024], bf16, tag="msk",
                                   name="msk_all")
        x_all = [persist.tile([128, NC, 512], bf16, tag=f"x{tb}",
                              name=f"x_all{tb}") for tb in range(NTB)]

        # x(0) + wk feed the first compute; then the rest
        nc.sync.dma_start(x_all[0][:],
                          xT[:, 0:512].rearrange("(c p) t -> p c t", p=128))
        nc.sync.dma_start(wk_all[:], wkT.rearrange("(c p) o -> p c o", p=128))
        nc.sync.dma_start(wv_all[:], wvT.rearrange("(c p) o -> p c o", p=128))
        nc.sync.dma_start(wq_all[:], wqT.rearrange("(c p) o -> p c o", p=128))
        for tb in range(1, NTB):
            nc.sync.dma_start(
                x_all[tb][:],
                xT[:, tb * 512:(tb + 1) * 512].rearrange("(c p) t -> p c t", p=128))
        nc.sync.dma_start(wo_all[:], woT.rearrange("(o p) c -> p o c", p=128))
        if npat:
            nc.sync.dma_start(msk_all[:], maskT.rearrange("s p q -> p s q"))
        for t in range(JT):
            nc.vector.memset(v_sb[t][:], 1.0)

        def proj_pair(w_all, dst, o0, tb, kind):
            """Two [128,512] projection chunks (o0, o0+1) in one ring slot."""
            tbs = slice(tb * 512, (tb + 1) * 512)
            ps = psS.tile([128, 1024], f32, tag="sS", name=f"pj_{kind}_{o0}_{tb}")
            for half in range(2):
                o = o0 + half
                hp = ps[:, half * 512:(half + 1) * 512]
                for c in range(NC):
                    nc.tensor.matmul(hp, w_all[:, c, o * 128:(o + 1) * 128],
                                     x_all[tb][:, c, :], start=(c == 0),
                                     stop=(c == NC - 1))
            for half in range(2):
                o = o0 + half
                hp = ps[:, half * 512:(half + 1) * 512]
                if kind == "q":
                    nc.scalar.copy(dst[o][:, tbs], hp)
                else:
                    nc.vector.tensor_copy(dst[o][:, tbs], hp)

        def vproj_pair(sub0, tb):
            """Two V chunks (sub0, sub0+1): psv [128 t, 512 od] halves."""
            ps = psS.tile([128, 1024], f32, tag="sS", name=f"pj_v_{sub0}_{tb}")
            for half in range(2):
                sub = sub0 + half
                hp = ps[:, half * 512:(half + 1) * 512]
                for c in range(NC):
                    nc.tensor.matmul(hp, x_all[tb][:, c, sub * 128:(sub + 1) * 128],
                                     wv_all[:, c, :], start=(c == 0),
                                     stop=(c == NC - 1))
            for half in range(2):
                t = tb * 4 + sub0 + half
                hp = ps[:, half * 512:(half + 1) * 512]
                v3 = v_sb[t][:].rearrange("p (h e) -> p h e", e=E)
                ps3 = hp.rearrange("p (h e) -> p h e", e=D)
                nc.vector.tensor_copy(v3[:, :, 0:D], ps3)

        def emit_oproj(qc):
            qq = slice(qc * 128, (qc + 1) * 128)
            psy = psS.tile([128, C], f32, tag="sS", name=f"psy_{qc}")
            for half in range(2):
                cs = slice(half * 512, (half + 1) * 512)
                for o in range(NO):
                    nc.tensor.matmul(psy[:, cs], aT_sb[o][:, qq],
                                     wo_all[:, o, cs],
                                     start=(o == 0), stop=(o == NO - 1))
            ys = pys.tile([128, C], f32, tag="ys", name=f"ys_{qc}")
            nc.vector.tensor_copy(ys[:], psy[:])
            nc.sync.dma_start(y[qq, :], ys[:])

        def run_filler(fl):
            if fl[0] == "psy":
                emit_oproj(fl[1])
            elif fl[0] == "q":
                proj_pair(wq_all, qT_sb, fl[1], fl[2], "q")
            elif fl[0] == "k":
                proj_pair(wk_all, kT_sb, fl[1], fl[2], "k")
            else:
                vproj_pair(fl[1], fl[2])

        # ======== front: K/V/Q of block 0 only ========
        for fl in [("k", 0, 0), ("k", 2, 0), ("v", 0, 0), ("v", 2, 0),
                   ("q", 0, 0), ("q", 2, 0)]:
            run_filler(fl)

        # ======== attention, everything else interleaved as fillers ========
        pending = []   # o_proj 128-query chunks awaiting emission
        for ib in range(IBN):
            js = [jt for jt in range(JT) if kinds[jt][ib] != 0]
            nj = len(js)
            fillq = []
            if ib + 1 < NTB:
                fillq.extend([("q", 0, ib + 1), ("q", 2, ib + 1)])
            if pending:
                fillq.extend(("psy", qc) for qc in pending)
                pending.clear()
            if ib + 1 < NTB:
                fillq.extend([("k", 0, ib + 1), ("k", 2, ib + 1),
                              ("v", 0, ib + 1), ("v", 2, ib + 1)])

            for p in range(NP):
                hA, hB = 2 * p, 2 * p + 1
                poA = po.tile([E, 512], f32, tag=f"poA{p % 2}",
                              name=f"poA_{ib}_{p}")
                poB = po.tile([E, 512], f32, tag=f"poB{p % 2}",
                              name=f"poB_{ib}_{p}")

                trims = []
                for jt in js:
                    tr = mixtrim[jt][ib] if kinds[jt][ib] == 2 else 0
                    trims.append(tr)
                trims[0] = 0  # first block must cover the full accumulator

                def emit_qk(cidx):
                    jt = js[cidx]
                    tr = trims[cidx]
                    jj = slice(jt * 128, (jt + 1) * 128)
                    qq = slice(ib * 512 + tr, (ib + 1) * 512)
                    sAB = psS.tile([128, 1024], f32, tag="sS",
                                   name=f"s_{ib}_{p}_{cidx}")
                    nc.tensor.matmul(sAB[:, tr:512], kT_sb[p][0:64, jj],
                                     qT_sb[p][0:64, qq],
                                     start=True, stop=True,
                                     tile_position=(0, 0))
                    nc.tensor.matmul(sAB[:, 512 + tr:1024], kT_sb[p][64:128, jj],
                                     qT_sb[p][64:128, qq],
                                     start=True, stop=True,
                                     tile_position=(64, 0))
                    pAB = ppp.tile([128, 1024], bf16, tag="pP",
                                   name=f"pp_{ib}_{p}_{cidx}")
                    if tr:
                        s3 = sAB.rearrange("p (h q) -> p h q", q=512)
                        p3 = pAB[:].rearrange("p (h q) -> p h q", q=512)
                        nc.scalar.activation(p3[:, :, tr:512], s3[:, :, tr:512],
                                             EXP, scale=SCALE)
                    else:
                        nc.scalar.activation(pAB[:], sAB[:], EXP, scale=SCALE)
                    if kinds[jt][ib] == 2:
                        p3 = pAB[:].rearrange("p (h q) -> p h q", q=512)
                        m3 = msk_all[:, mixpat[jt][ib], :].rearrange(
                            "p (h q) -> p h q", q=512)
                        nc.vector.tensor_mul(p3[:, :, tr:512], p3[:, :, tr:512],
                                             m3[:, :, tr:512])
                    return pAB

                def emit_av(cidx, pAB):
                    jt = js[cidx]
                    tr = trims[cidx]
                    first = (cidx == 0)
                    last = (cidx == nj - 1)
                    nc.tensor.matmul(poA[:, tr:512],
                                     v_sb[jt][:, hA * E:(hA + 1) * E],
                                     pAB[:, tr:512], start=first, stop=last,
                                     skip_group_check=True)
                    nc.tensor.matmul(poB[:, tr:512],
                                     v_sb[jt][:, hB * E:(hB + 1) * E],
                                     pAB[:, 512 + tr:1024], start=first,
                                     stop=last, skip_group_check=True)

                # software pipeline: QK0, QK1, filler, AV0, QK2, AV1, ...
                # extra fillers every few blocks keep PE ahead of the exp
                # stream on long rows
                pabs = {}
                pabs[0] = emit_qk(0)
                if nj > 1:
                    pabs[1] = emit_qk(1)
                nfill = 2 if nj > 8 else 1
                for _ in range(nfill):
                    if fillq:
                        run_filler(fillq.pop(0))
                for cidx in range(nj):
                    if cidx + 2 < nj:
                        pabs[cidx + 2] = emit_qk(cidx + 2)
                    if cidx and cidx % 5 == 0 and fillq:
                        run_filler(fillq.pop(0))
                    emit_av(cidx, pabs.pop(cidx))
                # softmax normalization: denominators sit in row 64 (ones col)
                dnA = pnrm.tile([1, 512], f32, tag="dnA", name=f"dnA_{ib}_{p}")
                dnB = pnrm.tile([1, 512], f32, tag="dnB", name=f"dnB_{ib}_{p}")
                nc.vector.tensor_copy(dnA[:], poA[64:65, :])
                nc.vector.tensor_copy(dnB[:], poB[64:65, :])
                rrA = pnrm.tile([1, 512], f32, tag="rrA", name=f"rrA_{ib}_{p}")
                rrB = pnrm.tile([1, 512], f32, tag="rrB", name=f"rrB_{ib}_{p}")
                nc.vector.reciprocal_approx_fast(rrA[:], dnA[:])
                nc.vector.reciprocal_approx_fast(rrB[:], dnB[:])
                bcA = pnrm.tile([64, 512], f32, tag="bcA", name=f"bcA_{ib}_{p}")
                bcB = pnrm.tile([64, 512], f32, tag="bcB", name=f"bcB_{ib}_{p}")
                nc.gpsimd.partition_broadcast(bcA[:], rrA[:])
                nc.gpsimd.partition_broadcast(bcB[:], rrB[:])
                ii = slice(ib * 512, (ib + 1) * 512)
                nc.vector.tensor_mul(aT_sb[p][0:64, ii], poA[0:64, :], bcA[:])
                stgB = pnrm.tile([64, 512], bf16, tag="stgB", name=f"stgB_{ib}_{p}")
                nc.vector.tensor_mul(stgB[:], poB[0:64, :], bcB[:])
                nc.sync.dma_start(aT_sb[p][64:128, ii], stgB[:])
            for fl in fillq:   # drain any leftovers
                run_filler(fl)
            pending.extend(range(ib * 4, ib * 4 + 4))
        for qc in pending:
            emit_oproj(qc)

        if debug:
            for o in range(NO):
                nc.sync.dma_start(dbg["qT"][o], qT_sb[o][:])
                nc.sync.dma_start(dbg["kT"][o], kT_sb[o][:])
                nc.sync.dma_start(dbg["aT"][o], aT_sb[o][:])
            for t in range(JT):
                nc.sync.dma_start(dbg["v"][t], v_sb[t][:])

    nc.compile()
    return nc


def classify_mask(mask2d, T):
    """mask2d: [T, T] (i=query rows, j=key cols).

    Returns kinds[jt][ib] in {0 empty, 1 full, 2 mixed}, mixpat[jt][ib]
    (index into the deduped pattern list), mixtrim[jt][ib] (count of leading
    query columns that are entirely masked, so QK/exp/AV can skip them), and
    patterns [n, 128, 1024] float32 (key-major tiles, duplicated along the
    free axis so one multiply covers both packed heads)."""
    JT, IBN = T // 128, T // 512
    kinds = [[0] * IBN for _ in range(JT)]
    mixpat = [[-1] * IBN for _ in range(JT)]
    mixtrim = [[0] * IBN for _ in range(JT)]
    patterns = []
    seen = {}
    for jt in range(JT):
        for ib in range(IBN):
            blk = mask2d[ib * 512:(ib + 1) * 512, jt * 128:(jt + 1) * 128]
            if not blk.any():
                kinds[jt][ib] = 0
            elif blk.all():
                kinds[jt][ib] = 1
            else:
                kinds[jt][ib] = 2
                tileT = np.ascontiguousarray(blk.T.astype(np.float32))
                key = tileT.tobytes()
                if key not in seen:
                    seen[key] = len(patterns)
                    patterns.append(np.concatenate([tileT, tileT], axis=1))
                mixpat[jt][ib] = seen[key]
                colvalid = tileT.any(axis=0)
                mixtrim[jt][ib] = int(np.argmax(colvalid))
    pat = np.stack(patterns) if patterns else None
    return kinds, mixpat, mixtrim, pat


B, T, C = 4, 2048, 1024
H, HD = 16, 64
G = 2
HL = H // G
OL = HL * HD

_cache = {}


def _prepare(x, mask, wq, wk, wv, wo):
    """Classify the mask, build (or reuse) the compiled kernel, and build
    the 8 per-core input maps."""
    bf = ml_dtypes.bfloat16
    x = np.asarray(x, dtype=np.float32)
    mask = np.asarray(mask)
    wq = np.asarray(wq, dtype=np.float32)
    wk = np.asarray(wk, dtype=np.float32)
    wv = np.asarray(wv, dtype=np.float32)
    wo = np.asarray(wo, dtype=np.float32)

    mask2d = mask.reshape(mask.shape[-2], mask.shape[-1])
    kinds, mixpat, mixtrim, pat = classify_mask(mask2d, T)
    npat = 0 if pat is None else len(pat)
    pat_bf = None if pat is None else pat.astype(bf)

    key = (tuple(tuple(r) for r in kinds), tuple(tuple(r) for r in mixpat),
           tuple(tuple(r) for r in mixtrim))
    if key not in _cache:
        _cache[key] = build(T=T, C=C, HL=HL, D=HD, kinds=kinds, mixpat=mixpat,
                            mixtrim=mixtrim, npat=npat, n_cores=8)
    nc = _cache[key]

    in_maps = []
    for b in range(B):
        for g in range(G):
            m = {
                "xT": np.ascontiguousarray(x[b].T).astype(bf),
                "wqT": np.ascontiguousarray(wq[g * OL:(g + 1) * OL, :].T).astype(bf),
                "wkT": np.ascontiguousarray(wk[g * OL:(g + 1) * OL, :].T).astype(bf),
                "wvT": np.ascontiguousarray(wv[g * OL:(g + 1) * OL, :].T).astype(bf),
                "woT": np.ascontiguousarray(wo[:, g * OL:(g + 1) * OL].T).astype(bf),
            }
            if npat:
                m["maskT"] = pat_bf
            in_maps.append(m)
    return nc, in_maps


def _gather(results):
    out = np.empty((B, T, C), np.float32)
    for b in range(B):
        out[b] = results[2 * b]["y"] + results[2 * b + 1]["y"]
    return out


def kernel(x, mask, wq, wk, wv, wo):
    from concourse import bass_utils
    nc, in_maps = _prepare(x, mask, wq, wk, wv, wo)
    res = bass_utils.run_bass_kernel_spmd(nc, in_maps, core_ids=list(range(8)))
    return _gather(res.results)


# revision 22
# speedup vs baseline: 1.4017x; 1.0248x over previous
"""Self-contained TRN2 Bass kernel for causal multi-head attention.

Problem: x[4,2048,1024], causal mask, wq/wk/wv/wo [1024,1024], H=16, HD=64.
Sharding: 8 NeuronCores = 4 batches x 2 head-groups (8 heads each).
Each core computes Q/K/V projections for its head group, causal attention
(block-skipped via host mask classification), and a partial o_proj; the two
partial outputs per batch are summed on host (the tensor-parallel
all-reduce of the unshard step).

v5: per-block fused schedule. Only Q/K/V of token-block 0 run up front
(~20us); the projections of block tb+1 and the o_proj of block ib-1 are
"fillers" interleaved into attention(ib)'s head-pair loops, keeping the PE
busy through every exp-latency window while the Act engine streams the
softmax exps nearly end-to-end. Everything is bf16 except PSUM accumulation
and the softmax normalization. One PSUM plan serves the whole kernel:
a 2-deep [128,1024] ring (score tiles, o_proj psy tiles, projection
pair-chunks) + 4 single-bank po tiles (two alternating pairs of packed-head
AV accumulators). Inputs are packed so each weight/x-block/mask set is one
DMA. Mixed causal blocks skip their fully-masked leading query columns in
QK/exp/mask/AV (diagonal trimming).
"""
import sys
sys.path.insert(0, "/opt/trn_rl_repo")

import numpy as np
import ml_dtypes

from contextlib import ExitStack

import concourse.bass as bass
import concourse.mybir as mybir
import concourse.tile as tile
from concourse import bacc

f32 = mybir.dt.float32
bf16 = mybir.dt.bfloat16
EXP = mybir.ActivationFunctionType.Exp


def build(T=2048, C=1024, HL=8, D=64, kinds=None, mixpat=None, mixtrim=None,
          npat=0, n_cores=8, debug=False):
    OL = HL * D
    JT = T // 128
    IBN = T // 512
    NC = C // 128
    NO = OL // 128
    NTB = T // 512
    NP = HL // 2
    SCALE = 1.0 / float(D) ** 0.5
    E = D + 1  # v columns per head incl. ones column

    assert kinds is not None

    nc = bacc.Bacc("TRN2", target_bir_lowering=False, debug=False,
                   num_devices=n_cores)

    xT = nc.dram_tensor("xT", [C, T], bf16, kind="ExternalInput").ap()
    wqT = nc.dram_tensor("wqT", [C, OL], bf16, kind="ExternalInput").ap()
    wkT = nc.dram_tensor("wkT", [C, OL], bf16, kind="ExternalInput").ap()
    wvT = nc.dram_tensor("wvT", [C, OL], bf16, kind="ExternalInput").ap()
    woT = nc.dram_tensor("woT", [OL, C], bf16, kind="ExternalInput").ap()
    maskT = None
    if npat:
        maskT = nc.dram_tensor("maskT", [npat, 128, 1024], bf16,
                               kind="ExternalInput").ap()
    y = nc.dram_tensor("y", [T, C], f32, kind="ExternalOutput").ap()
    dbg = {}
    if debug:
        dbg["qT"] = nc.dram_tensor("dbg_qT", [NO, 128, T], bf16, kind="ExternalOutput").ap()
        dbg["kT"] = nc.dram_tensor("dbg_kT", [NO, 128, T], bf16, kind="ExternalOutput").ap()
        dbg["v"] = nc.dram_tensor("dbg_v", [JT, 128, HL * E], bf16, kind="ExternalOutput").ap()
        dbg["aT"] = nc.dram_tensor("dbg_aT", [NO, 128, T], bf16, kind="ExternalOutput").ap()

    with tile.TileContext(nc) as tc, ExitStack() as ctx:
        # ---- pools live for the whole kernel ----
        persist = ctx.enter_context(tc.tile_pool(name="persist", bufs=1))
        ppp = ctx.enter_context(tc.tile_pool(name="ppp", bufs=4))
        pnrm = ctx.enter_context(tc.tile_pool(name="pnrm", bufs=2))
        pys = ctx.enter_context(tc.tile_pool(name="pys", bufs=2))
        psS = ctx.enter_context(tc.tile_pool(name="psS", bufs=2, space="PSUM"))
        po = ctx.enter_context(tc.tile_pool(name="po", bufs=1, space="PSUM"))

        qT_sb = [persist.tile([128, T], bf16, tag=f"qT{o}", name=f"qT{o}")
                 for o in range(NO)]
        kT_sb = [persist.tile([128, T], bf16, tag=f"kT{o}", name=f"kT{o}")
                 for o in range(NO)]
        v_sb = [persist.tile([128, HL * E], bf16, tag=f"v{t}", name=f"v{t}")
                for t in range(JT)]
        aT_sb = [persist.tile([128, T], bf16, tag=f"aT{o}", name=f"aT{o}")
                 for o in range(NO)]
        # packed: one DMA per tensor
        wq_all = persist.tile([128, NC, OL], bf16, tag="wq", name="wq_all")
        wk_all = persist.tile([128, NC, OL], bf16, tag="wk", name="wk_all")
        wv_all = persist.tile([128, NC, OL], bf16, tag="wv", name="wv_all")
        wo_all = persist.tile([128, NO, C], bf16, tag="wo", name="wo_all")
        msk_all = None
        if npat:
            msk_all = persist.tile([128, npat, 1024], bf16, tag="msk",
                                   name="msk_all")
        x_all = [persist.tile([128, NC, 512], bf16, tag=f"x{tb}",
                              name=f"x_all{tb}") for tb in range(NTB)]

        # x(0) + wk feed the first compute: split into chunks so several DMA
        # engines run in parallel; later loads are latency-insensitive
        x0r = xT[:, 0:512].rearrange("(c p) t -> p c t", p=128)
        wkr = wkT.rearrange("(c p) o -> p c o", p=128)
        for h in range(4):
            cs = slice(2 * h, 2 * h + 2)
            nc.sync.dma_start(x_all[0][:, cs, :], x0r[:, cs, :])
            nc.sync.dma_start(wk_all[:, cs, :], wkr[:, cs, :])
        wvr = wvT.rearrange("(c p) o -> p c o", p=128)
        wqr = wqT.rearrange("(c p) o -> p c o", p=128)
        for h in range(2):
            cs = slice(4 * h, 4 * h + 4)
            nc.sync.dma_start(wv_all[:, cs, :], wvr[:, cs, :])
            nc.sync.dma_start(wq_all[:, cs, :], wqr[:, cs, :])
        for tb in range(1, NTB):
            xr = xT[:, tb * 512:(tb + 1) * 512].rearrange("(c p) t -> p c t", p=128)
            for h in range(2):
                cs = slice(4 * h, 4 * h + 4)
                nc.sync.dma_start(x_all[tb][:, cs, :], xr[:, cs, :])
        nc.sync.dma_start(wo_all[:], woT.rearrange("(o p) c -> p o c", p=128))
        if npat:
            nc.sync.dma_start(msk_all[:], maskT.rearrange("s p q -> p s q"))
        for t in range(JT):
            nc.vector.memset(v_sb[t][:], 1.0)

        def proj_pair(w_all, dst, o0, tb, kind):
            """Two [128,512] projection chunks (o0, o0+1) in one ring slot."""
            tbs = slice(tb * 512, (tb + 1) * 512)
            ps = psS.tile([128, 1024], f32, tag="sS", name=f"pj_{kind}_{o0}_{tb}")
            for half in range(2):
                o = o0 + half
                hp = ps[:, half * 512:(half + 1) * 512]
                for c in range(NC):
                    nc.tensor.matmul(hp, w_all[:, c, o * 128:(o + 1) * 128],
                                     x_all[tb][:, c, :], start=(c == 0),
                                     stop=(c == NC - 1))
            for half in range(2):
                o = o0 + half
                hp = ps[:, half * 512:(half + 1) * 512]
                nc.vector.tensor_copy(dst[o][:, tbs], hp)

        def vproj_pair(sub0, tb):
            """Two V chunks (sub0, sub0+1): psv [128 t, 512 od] halves."""
            ps = psS.tile([128, 1024], f32, tag="sS", name=f"pj_v_{sub0}_{tb}")
            for half in range(2):
                sub = sub0 + half
                hp = ps[:, half * 512:(half + 1) * 512]
                for c in range(NC):
                    nc.tensor.matmul(hp, x_all[tb][:, c, sub * 128:(sub + 1) * 128],
                                     wv_all[:, c, :], start=(c == 0),
                                     stop=(c == NC - 1))
            for half in range(2):
                t = tb * 4 + sub0 + half
                hp = ps[:, half * 512:(half + 1) * 512]
                v3 = v_sb[t][:].rearrange("p (h e) -> p h e", e=E)
                ps3 = hp.rearrange("p (h e) -> p h e", e=D)
                nc.vector.tensor_copy(v3[:, :, 0:D], ps3)

        def emit_oproj(qc, direct=False):
            qq = slice(qc * 128, (qc + 1) * 128)
            psy = psS.tile([128, C], f32, tag="sS", name=f"psy_{qc}")
            for half in range(2):
                cs = slice(half * 512, (half + 1) * 512)
                for o in range(NO):
                    nc.tensor.matmul(psy[:, cs], aT_sb[o][:, qq],
                                     wo_all[:, o, cs],
                                     start=(o == 0), stop=(o == NO - 1))
            ys = pys.tile([128, C], f32, tag="ys", name=f"ys_{qc}")
            if direct:
                # tail chunks: DVE may still be busy with the last norm; the
                # scalar engine is idle once the exp stream ends
                nc.scalar.copy(ys[:], psy[:])
            else:
                nc.vector.tensor_copy(ys[:], psy[:])
            nc.gpsimd.dma_start(y[qq, :], ys[:])

        def run_filler(fl):
            if fl[0] == "psy":
                emit_oproj(fl[1])
            elif fl[0] == "q":
                proj_pair(wq_all, qT_sb, fl[1], fl[2], "q")
            elif fl[0] == "k":
                proj_pair(wk_all, kT_sb, fl[1], fl[2], "k")
            else:
                vproj_pair(fl[1], fl[2])

        # ======== front: K/V/Q of block 0 only ========
        for fl in [("k", 0, 0), ("k", 2, 0), ("v", 0, 0), ("v", 2, 0),
                   ("q", 0, 0), ("q", 2, 0)]:
            run_filler(fl)

        # ======== attention, everything else interleaved as fillers ========
        pending = []   # o_proj 128-query chunks awaiting emission
        for ib in range(IBN):
            js = [jt for jt in range(JT) if kinds[jt][ib] != 0]
            nj = len(js)
            fillq = []
            if ib + 1 < NTB:
                fillq.extend([("q", 0, ib + 1), ("q", 2, ib + 1),
                              ("k", 0, ib + 1), ("k", 2, ib + 1),
                              ("v", 0, ib + 1), ("v", 2, ib + 1)])
            # defer most o_proj chunks toward the last (longest, exp-paced)
            # block rows, where the PE has exp-latency bubbles to fill
            take = 2 if ib + 1 < IBN else len(pending)
            fillq.extend(("psy", qc) for qc in pending[:take])
            del pending[:take]

            for p in range(NP):
                hA, hB = 2 * p, 2 * p + 1
                poA = po.tile([E, 512], f32, tag=f"poA{p % 2}",
                              name=f"poA_{ib}_{p}")
                poB = po.tile([E, 512], f32, tag=f"poB{p % 2}",
                              name=f"poB_{ib}_{p}")

                trims = []
                for jt in js:
                    tr = mixtrim[jt][ib] if kinds[jt][ib] == 2 else 0
                    trims.append(tr)
                trims[0] = 0  # first block must cover the full accumulator

                def emit_qk(cidx):
                    jt = js[cidx]
                    tr = trims[cidx]
                    jj = slice(jt * 128, (jt + 1) * 128)
                    qq = slice(ib * 512 + tr, (ib + 1) * 512)
                    sAB = psS.tile([128, 1024], f32, tag="sS",
                                   name=f"s_{ib}_{p}_{cidx}")
                    nc.tensor.matmul(sAB[:, tr:512], kT_sb[p][0:64, jj],
                                     qT_sb[p][0:64, qq],
                                     start=True, stop=True,
                                     tile_position=(0, 0))
                    nc.tensor.matmul(sAB[:, 512 + tr:1024], kT_sb[p][64:128, jj],
                                     qT_sb[p][64:128, qq],
                                     start=True, stop=True,
                                     tile_position=(64, 0))
                    pAB = ppp.tile([128, 1024], bf16, tag="pP",
                                   name=f"pp_{ib}_{p}_{cidx}")
                    if tr:
                        s3 = sAB.rearrange("p (h q) -> p h q", q=512)
                        p3 = pAB[:].rearrange("p (h q) -> p h q", q=512)
                        nc.scalar.activation(p3[:, :, tr:512], s3[:, :, tr:512],
                                             EXP, scale=SCALE)
                    else:
                        nc.scalar.activation(pAB[:], sAB[:], EXP, scale=SCALE)
                    if kinds[jt][ib] == 2:
                        p3 = pAB[:].rearrange("p (h q) -> p h q", q=512)
                        m3 = msk_all[:, mixpat[jt][ib], :].rearrange(
                            "p (h q) -> p h q", q=512)
                        nc.vector.tensor_mul(p3[:, :, tr:512], p3[:, :, tr:512],
                                             m3[:, :, tr:512])
                    return pAB

                def emit_av(cidx, pAB):
                    jt = js[cidx]
                    tr = trims[cidx]
                    first = (cidx == 0)
                    last = (cidx == nj - 1)
                    nc.tensor.matmul(poA[:, tr:512],
                                     v_sb[jt][:, hA * E:(hA + 1) * E],
                                     pAB[:, tr:512], start=first, stop=last,
                                     skip_group_check=True)
                    nc.tensor.matmul(poB[:, tr:512],
                                     v_sb[jt][:, hB * E:(hB + 1) * E],
                                     pAB[:, 512 + tr:1024], start=first,
                                     stop=last, skip_group_check=True)

                # software pipeline: QK0, QK1, filler, AV0, QK2, AV1, ...
                # extra fillers every few blocks keep PE ahead of the exp
                # stream on long rows
                pabs = {}
                pabs[0] = emit_qk(0)
                if nj > 1:
                    pabs[1] = emit_qk(1)
                nfill = 2 if nj > 8 else 1
                for _ in range(nfill):
                    if fillq:
                        run_filler(fillq.pop(0))
                for cidx in range(nj):
                    if cidx + 2 < nj:
                        pabs[cidx + 2] = emit_qk(cidx + 2)
                    if cidx and cidx % 5 == 0 and fillq:
                        run_filler(fillq.pop(0))
                    emit_av(cidx, pabs.pop(cidx))
                # softmax normalization: denominators sit in row 64 (ones col)
                dnA = pnrm.tile([1, 512], f32, tag="dnA", name=f"dnA_{ib}_{p}")
                dnB = pnrm.tile([1, 512], f32, tag="dnB", name=f"dnB_{ib}_{p}")
                nc.vector.tensor_copy(dnA[:], poA[64:65, :])
                nc.vector.tensor_copy(dnB[:], poB[64:65, :])
                rrA = pnrm.tile([1, 512], f32, tag="rrA", name=f"rrA_{ib}_{p}")
                rrB = pnrm.tile([1, 512], f32, tag="rrB", name=f"rrB_{ib}_{p}")
                nc.vector.reciprocal_approx_fast(rrA[:], dnA[:])
                nc.vector.reciprocal_approx_fast(rrB[:], dnB[:])
                bcA = pnrm.tile([64, 512], f32, tag="bcA", name=f"bcA_{ib}_{p}")
                bcB = pnrm.tile([64, 512], f32, tag="bcB", name=f"bcB_{ib}_{p}")
                nc.gpsimd.partition_broadcast(bcA[:], rrA[:])
                nc.gpsimd.partition_broadcast(bcB[:], rrB[:])
                ii = slice(ib * 512, (ib + 1) * 512)
                nc.vector.tensor_mul(aT_sb[p][0:64, ii], poA[0:64, :], bcA[:])
                stgB = pnrm.tile([64, 512], bf16, tag="stgB", name=f"stgB_{ib}_{p}")
                nc.vector.tensor_mul(stgB[:], poB[0:64, :], bcB[:])
                nc.gpsimd.dma_start(aT_sb[p][64:128, ii], stgB[:])
            for fl in fillq:   # drain any leftovers
                run_filler(fl)
            pending.extend(range(ib * 4, ib * 4 + 4))
        for qc in pending:
            emit_oproj(qc, direct=True)

        if debug:
            for o in range(NO):
                nc.sync.dma_start(dbg["qT"][o], qT_sb[o][:])
                nc.sync.dma_start(dbg["kT"][o], kT_sb[o][:])
                nc.sync.dma_start(dbg["aT"][o], aT_sb[o][:])
            for t in range(JT):
                nc.sync.dma_start(dbg["v"][t], v_sb[t][:])

    nc.compile()
    return nc


def classify_mask(mask2d, T):
    """mask2d: [T, T] (i=query rows, j=key cols).

    Returns kinds[jt][ib] in {0 empty, 1 full, 2 mixed}, mixpat[jt][ib]
    (index into the deduped pattern list), mixtrim[jt][ib] (count of leading
    query columns that are entirely masked, so QK/exp/AV can skip them), and
    patterns [n, 128, 1024] float32 (key-major tiles, duplicated along the
    free axis so one multiply covers both packed heads)."""
    JT, IBN = T // 128, T // 512
    kinds = [[0] * IBN for _ in range(JT)]
    mixpat = [[-1] * IBN for _ in range(JT)]
    mixtrim = [[0] * IBN for _ in range(JT)]
    patterns = []
    seen = {}
    for jt in range(JT):
        for ib in range(IBN):
            blk = mask2d[ib * 512:(ib + 1) * 512, jt * 128:(jt + 1) * 128]
            if not blk.any():
                kinds[jt][ib] = 0
            elif blk.all():
                kinds[jt][ib] = 1
            else:
                kinds[jt][ib] = 2
                tileT = np.ascontiguousarray(blk.T.astype(np.float32))
                key = tileT.tobytes()
                if key not in seen:
                    seen[key] = len(patterns)
                    patterns.append(np.concatenate([tileT, tileT], axis=1))
                mixpat[jt][ib] = seen[key]
                colvalid = tileT.any(axis=0)
                mixtrim[jt][ib] = int(np.argmax(colvalid))
    pat = np.stack(patterns) if patterns else None
    return kinds, mixpat, mixtrim, pat


B, T, C = 4, 2048, 1024
H, HD = 16, 64
G = 2
HL = H // G
OL = HL * HD

_cache = {}


def _prepare(x, mask, wq, wk, wv, wo):
    """Classify the mask, build (or reuse) the compiled kernel, and build
    the 8 per-core input maps."""
    bf = ml_dtypes.bfloat16
    x = np.asarray(x, dtype=np.float32)
    mask = np.asarray(mask)
    wq = np.asarray(wq, dtype=np.float32)
    wk = np.asarray(wk, dtype=np.float32)
    wv = np.asarray(wv, dtype=np.float32)
    wo = np.asarray(wo, dtype=np.float32)

    mask2d = mask.reshape(mask.shape[-2], mask.shape[-1])
    kinds, mixpat, mixtrim, pat = classify_mask(mask2d, T)
    npat = 0 if pat is None else len(pat)
    pat_bf = None if pat is None else pat.astype(bf)

    key = (tuple(tuple(r) for r in kinds), tuple(tuple(r) for r in mixpat),
           tuple(tuple(r) for r in mixtrim))
    if key not in _cache:
        _cache[key] = build(T=T, C=C, HL=HL, D=HD, kinds=kinds, mixpat=mixpat,
                            mixtrim=mixtrim, npat=npat, n_cores=8)
    nc = _cache[key]

    in_maps = []
    for b in range(B):
        for g in range(G):
            m = {
                "xT": np.ascontiguousarray(x[b].T).astype(bf),
                "wqT": np.ascontiguousarray(wq[g * OL:(g + 1) * OL, :].T).astype(bf),
                "wkT": np.ascontiguousarray(wk[g * OL:(g + 1) * OL, :].T).astype(bf),
                "wvT": np.ascontiguousarray(wv[g * OL:(g + 1) * OL, :].T).astype(bf),
                "woT": np.ascontiguousarray(wo[:, g * OL:(g + 1) * OL].T).astype(bf),
            }
            if npat:
                m["maskT"] = pat_bf
            in_maps.append(m)
    return nc, in_maps


def _gather(results):
    out = np.empty((B, T, C), np.float32)
    for b in range(B):
        out[b] = results[2 * b]["y"] + results[2 * b + 1]["y"]
    return out


def kernel(x, mask, wq, wk, wv, wo):
    from concourse import bass_utils
    nc, in_maps = _prepare(x, mask, wq, wk, wv, wo)
    res = bass_utils.run_bass_kernel_spmd(nc, in_maps, core_ids=list(range(8)))
    return _gather(res.results)
